# revision 13
# baseline (speedup 1.0000x reference)
"""Trainium2 Bass kernel for nn_Cross_Attention (triplet-pool cross-attention gating).

Math (per sample b):
  pools:  Shw[h,w]=max_c x,  Sch[c,h]=max_w x,  Scw[c,w]=max_h x
  3 branches of flat-softmax cross attention between pools -> y12,y13 [h,w],
  y21,y23 [c,h], y31,y32 [c,w]
  training-mode BatchNorm over the *global* batch (cross-core allreduce of
  sum/sumsq), sigmoid gates, and finally
  out = x * (g12*g13)[h,w] * (g21*g23)[c,h] * (g31*g32)[c,w] + x
      = x * (1 + A[h,w]*B[c,h]*Cg[c,w])

Sharding: batch-parallel, 2 samples per core on 8 cores; only the BN batch
stats cross cores (AllReduce of a [128,20] tile).

Engine split (v2): x is converted to a resident bf16 copy on the Act engine;
all three pools come from that copy -- Sch/Scw via DVE tensor_tensor max
trees (bf16 packed operands run the DVE at 2x), Shw via a gpsimd pair-max +
cross-partition tensor_reduce (axis=C) with a small SBUF->SBUF DMA to lay the
rows out partition=h.  The apply phase is three 2x TT ops plus a 4x
tensor-scalar (+1): m = (A*C)[mid-bcast] * B_exp, out = (m+1)*x, where B_exp
is materialised by the Act engine (innermost-stride-0 operands would drop the
DVE to 1x).
"""

import numpy as np

import concourse.bacc as bacc
import concourse.mybir as mybir
import concourse.tile as tile
from concourse import masks

f32 = mybir.dt.float32
bf16 = mybir.dt.bfloat16
Alu = mybir.AluOpType
Act = mybir.ActivationFunctionType
X = mybir.AxisListType.X
CAX = mybir.AxisListType.C

NCORES = 8
S = 2          # samples per core
C, H, W = 256, 128, 128
CT = 2         # c tiles of 128
HC = 16        # h rows per chunk
NCH = H // HC  # 8
NP = 20        # bnp columns
EPS = 1e-5


def build_bass(n_cores: int, sync_start: bool = False, phases: str = "ABCDE"):
    """sync_start/phases are for timing probes only: sync_start prepends a
    tiny AllReduce so all cores start main work in lockstep (makes full
    device time visible to the marginal-time harness); phases truncates."""
    nc = bacc.Bacc("TRN2", target_bir_lowering=False, debug=False,
                   num_devices=n_cores)
    nb_tot = n_cores * S
    n1 = float(nb_tot * H * W)   # bn1 count
    ncn = float(nb_tot * H)      # bnc count (per channel)

    xs = nc.dram_tensor("xs", [S, C, H, W], f32, kind="ExternalInput").ap()
    bn1w = nc.dram_tensor("bn1_w", [1], f32, kind="ExternalInput").ap()
    bn1b = nc.dram_tensor("bn1_b", [1], f32, kind="ExternalInput").ap()
    bncw = nc.dram_tensor("bnc_w", [C], f32, kind="ExternalInput").ap()
    bncb = nc.dram_tensor("bnc_b", [C], f32, kind="ExternalInput").ap()
    outy = nc.dram_tensor("outy", [S, C, H, W], bf16,
                          kind="ExternalOutput").ap()

    ccin = nc.dram_tensor("ccin", [128, NP], f32).ap()
    ccout = nc.dram_tensor(
        "ccout", [128, NP], f32,
        addr_space="Shared" if n_cores > 1 else "Local").ap()
    adram = nc.dram_tensor("adram", [S, H * W], bf16).ap()
    srow = nc.dram_tensor("srow", [S, H * W], bf16).ap()
    sync_bufs = None
    if sync_start:
        sin = nc.dram_tensor("sin", [1, 1], f32).ap()
        sout = nc.dram_tensor(
            "sout", [1, 1], f32,
            addr_space="Shared" if n_cores > 1 else "Local").ap()
        sync_bufs = (sin, sout)

    with tile.TileContext(nc) as tc:
        _emit(nc, tc, n_cores, n1, ncn,
              xs, bn1w, bn1b, bncw, bncb, outy, ccin, ccout, adram, srow,
              sync_bufs, phases)
    nc.compile()
    return nc


def _emit(nc, tc, n_cores, n1, ncn,
          xs, bn1w, bn1b, bncw, bncb, outy, ccin, ccout, adram, srow,
          sync_bufs=None, phases="ABCDE"):
    import contextlib
    stack = contextlib.ExitStack()
    with stack:
        persist = stack.enter_context(tc.tile_pool(name="persist", bufs=1))
        maps = stack.enter_context(tc.tile_pool(name="maps", bufs=2))
        cols = stack.enter_context(tc.tile_pool(name="cols", bufs=4))
        keep = stack.enter_context(tc.tile_pool(name="keep", bufs=1))
        gscr = stack.enter_context(tc.tile_pool(name="gscr", bufs=4))

        # --- timing-only start barrier: a tiny AllReduce whose result is
        # loaded on the sync DMA queue, so every later HWDGE load (FIFO per
        # engine) waits until all cores have started this iteration ---
        if sync_bufs is not None:
            sin, sout = sync_bufs
            st0 = cols.tile([1, 1], f32, name="st0", tag="c0")
            nc.vector.memset(st0[:], 1.0)
            nc.sync.dma_start(sin, st0[:])
            nc.gpsimd.collective_compute(
                "AllReduce", Alu.add,
                replica_groups=[list(range(n_cores))],
                ins=[sin], outs=[sout])
            st1 = cols.tile([1, 1], f32, name="st1", tag="c0")
            nc.sync.dma_start(st1[:], sout)
        else:
            st1 = None

        # --- setup ---
        identity = persist.tile([128, 128], f32)
        masks.make_identity(nc, identity[:])
        identb = persist.tile([128, 128], bf16)
        nc.vector.tensor_copy(identb[:], identity[:])
        ones_r = persist.tile([1, 128], f32)
        nc.vector.memset(ones_r[:], 1.0)
        ones_c = persist.tile([128, 1], f32)
        nc.vector.memset(ones_c[:], 1.0)
        eps_col = persist.tile([128, 1], f32)
        nc.vector.memset(eps_col[:], EPS)
        wc2 = persist.tile([128, 2], f32)
        nc.sync.dma_start(wc2[:], bncw.rearrange("(t c) -> c t", c=128))
        bc2 = persist.tile([128, 2], f32)
        nc.sync.dma_start(bc2[:], bncb.rearrange("(t c) -> c t", c=128))
        bn1w_sb = persist.tile([1, 1], f32)
        nc.sync.dma_start(bn1w_sb[:], bn1w.unsqueeze(1))
        bn1b_sb = persist.tile([1, 1], f32)
        nc.sync.dma_start(bn1b_sb[:], bn1b.unsqueeze(1))
        wc8 = persist.tile([128, 8], f32)
        bc8 = persist.tile([128, 8], f32)
        for m in range(4):
            nc.vector.tensor_copy(wc8[:, m * 2:m * 2 + 2], wc2[:])
            nc.vector.tensor_copy(bc8[:, m * 2:m * 2 + 2], bc2[:])
        bnpS = []
        for s in range(S):
            t_ = persist.tile([128, NP], f32, name=f"bnpS{s}")
            nc.vector.memset(t_[:], 0.0)
            bnpS.append(t_)
        bnp = persist.tile([128, NP], f32)

        # per-sample persistent maps (bufs=2 -> one slot per sample)
        def smap(name, shape, bufs=None, dtype=f32):
            return [maps.tile(shape, dtype, name=f"{name}{s}", tag=name,
                              bufs=bufs)
                    for s in range(S)]

        xch = smap("xch", [128, CT * H], dtype=bf16)    # [c_loc, (t,h)]
        xcw = smap("xcw", [128, CT * W], dtype=bf16)    # [c_loc, (t,w)]
        xhwT = smap("xhwT", [128, H], dtype=bf16)       # [w, h]
        shw = smap("shw", [128, W], dtype=bf16)         # [h, w]
        y12 = smap("y12", [128, W])         # [h, w] f32
        y13 = smap("y13", [128, W])         # [h, w] f32
        # the four bnc y-maps live in one [128, (m,t)*128] tile per sample,
        # cols (m, t): m0=y21(br12) m1=y23(br23) m2=y31(br13) m3=y32(br23),
        # so the whole B/C gate batch is one sigmoid activation later
        Y8 = smap("Y8", [128, 8 * 128])

        def ymt(s, m, t):
            return Y8[s][:, (2 * m + t) * 128:(2 * m + t + 1) * 128]

        def ym(s, m):
            return Y8[s][:, 2 * m * 128:(2 * m + 2) * 128]
        agate = smap("agate", [128, W], dtype=bf16)     # [h, w]
        itc = {}   # invT cols [128,1] per (s, branch)
        it1 = {}   # invT [1,1] per (s, branch)

        # resident bf16 x: [c_loc, h, w] per (s, t) -- conversion on Act,
        # source for all pools and the apply phase
        xres = [[persist.tile([128, H, W], bf16,
                              name=f"xres{s}{t}", tag=f"xres{s}{t}")
                 for t in range(CT)] for s in range(S)]

        ps_stack = contextlib.ExitStack()
        with ps_stack:
            ptree = ps_stack.enter_context(tc.tile_pool(name="ptree", bufs=1))
            ppm = ps_stack.enter_context(tc.tile_pool(name="ppm", bufs=2))
            pbm = ps_stack.enter_context(tc.tile_pool(name="pbm", bufs=1))

            def bmap(name, shape, dtype=f32):
                return [pbm.tile(shape, dtype, name=f"{name}{s}", tag=name)
                        for s in range(S)]

            e12 = bmap("e12", [128, C], bf16)         # [w, c]
            e12t = bmap("e12t", [128, CT * W], bf16)  # [c_loc, (t,w)]
            e13 = bmap("e13", [128, C], bf16)         # [h, c]
            e13t = bmap("e13t", [128, CT * H], bf16)  # [c_loc, (t,h)]
            e23 = bmap("e23", [128, W], bf16)         # [h, w]
            e23t = bmap("e23t", [128, H], bf16)       # [w, h]
            y12T = bmap("y12T", [128, H])             # [w, h] f32
            ps_mm = ps_stack.enter_context(
                tc.tile_pool(name="ps_mm", bufs=2, space="PSUM"))
            ps_ty = ps_stack.enter_context(
                tc.tile_pool(name="ps_ty", bufs=2, space="PSUM"))

            # ---------------- pass 1: load, convert, pools ----------------
            def emit_pass1(s):
                HH = H // 2
                for half in range(2):
                    for t in range(CT):
                        # casting DMA (gpsimd-initiated SWDGE): f32 DRAM ->
                        # bf16 resident copy, half a c-tile per trigger to
                        # amortise the SWDGE fixed cost on the Pool queue
                        dst = xres[s][t][:, half * HH:(half + 1) * HH, :]
                        if st1 is not None:
                            # timing probe: force every load to wait for the
                            # start barrier via a WAW dep on the tile
                            nc.vector.tensor_copy(
                                xres[s][t][0:1, half * HH, 0:1], st1[:])
                        nc.gpsimd.dma_start(
                            dst, xs[s, t * 128:(t + 1) * 128,
                                    half * HH:(half + 1) * HH, :])
                    for k in range(half * (NCH // 2), (half + 1) * (NCH // 2)):
                        # Shw: pair-max over the two c tiles (alternating
                        # DVE/Pool), then max over the 128 c_loc partitions
                        # (Pool, axis=C), then a DRAM bounce to partition=h
                        pm = ppm.tile([128, HC, W], bf16, name=f"pm{s}{k}",
                                      tag="pm")
                        # neuronxcc rejects TensorTensor on Pool; DVE 2x
                        nc.vector.tensor_tensor(
                            out=pm[:],
                            in0=xres[s][0][:, k * HC:(k + 1) * HC, :],
                            in1=xres[s][1][:, k * HC:(k + 1) * HC, :],
                            op=Alu.max)
                        row = ppm.tile([1, HC * W], bf16, name=f"row{s}{k}",
                                       tag="row")
                        nc.gpsimd.tensor_reduce(out=row[:], in_=pm[:],
                                                axis=CAX, op=Alu.max)
                        nc.sync.dma_start(
                            srow[s, k * HC * W:(k + 1) * HC * W].unsqueeze(0),
                            row[:])
                # Shw rows land partition=h via a DRAM bounce
                nc.sync.dma_start(
                    shw[s][:], srow[s].rearrange("(h w) -> h w", h=H))

                # Sch / Scw via DVE bf16 max trees (2x packed mode).
                # Quarter-fold first so every scratch tile is <= 8KB and all
                # levels cycle through one shared 3-buffer tag.
                def tscr(shape, nm):
                    return ptree.tile(shape, bf16, name=nm, tag="tr", bufs=3)

                for t in range(CT):
                    xr = xres[s][t]
                    # Scw: fold h 128 -> 1
                    q1 = tscr([128, 32, W], f"cwq1{s}{t}")
                    nc.vector.tensor_tensor(out=q1[:], in0=xr[:, 0:32, :],
                                            in1=xr[:, 64:96, :], op=Alu.max)
                    q2 = tscr([128, 32, W], f"cwq2{s}{t}")
                    nc.vector.tensor_tensor(out=q2[:], in0=xr[:, 32:64, :],
                                            in1=xr[:, 96:128, :], op=Alu.max)
                    cur = tscr([128, 32, W], f"cwm32{s}{t}")
                    nc.vector.tensor_tensor(out=cur[:], in0=q1[:], in1=q2[:],
                                            op=Alu.max)
                    hs = 16
                    while hs >= 1:
                        if hs == 1:
                            dst_ap = xcw[s][:, t * W:(t + 1) * W].unsqueeze(1)
                        else:
                            nt = tscr([128, hs, W], f"cw{s}{t}{hs}")
                            dst_ap = nt[:]
                        nc.vector.tensor_tensor(
                            out=dst_ap, in0=cur[:, 0:hs, :],
                            in1=cur[:, hs:2 * hs, :], op=Alu.max)
                        if hs > 1:
                            cur = nt
                        hs //= 2
                    # Sch: fold w 128 -> 1
                    q1 = tscr([128, H, 32], f"chq1{s}{t}")
                    nc.vector.tensor_tensor(out=q1[:], in0=xr[:, :, 0:32],
                                            in1=xr[:, :, 64:96], op=Alu.max)
                    q2 = tscr([128, H, 32], f"chq2{s}{t}")
                    nc.vector.tensor_tensor(out=q2[:], in0=xr[:, :, 32:64],
                                            in1=xr[:, :, 96:128], op=Alu.max)
                    cur = tscr([128, H, 32], f"chm32{s}{t}")
                    nc.vector.tensor_tensor(out=cur[:], in0=q1[:], in1=q2[:],
                                            op=Alu.max)
                    ws = 16
                    while ws >= 1:
                        if ws == 1:
                            dst_ap = xch[s][:, t * H:(t + 1) * H].unsqueeze(2)
                        else:
                            nt = tscr([128, H, ws], f"ch{s}{t}{ws}")
                            dst_ap = nt[:]
                        nc.vector.tensor_tensor(
                            out=dst_ap, in0=cur[:, :, 0:ws],
                            in1=cur[:, :, ws:2 * ws], op=Alu.max)
                        if ws > 1:
                            cur = nt
                        ws //= 2

                # xhwT = transpose(shw)
                tp = ps_ty.tile([128, 128], bf16, name=f"shwT{s}", tag="tyb")
                nc.tensor.transpose(tp[:], shw[s][:], identb[:])
                nc.vector.tensor_copy(xhwT[s][:], tp[:])

            # ---------------- phase B: attention ----------------
            def psum_copy_to(dst, src_ps):
                nc.scalar.copy(dst, src_ps)

            def transpose_to(dst, src_sb, nblk, name):
                # src [128, nblk*128] bf16 -> dst [128, nblk*128] blockwise T
                for t in range(nblk):
                    tp = ps_mm.tile([128, 128], bf16, name=f"tp{name}{t}",
                                    tag="mmb")
                    nc.tensor.transpose(
                        tp[:], src_sb[:, t * 128:(t + 1) * 128], identb[:])
                    psum_copy_to(dst[:, t * 128:(t + 1) * 128], tp[:])

            def softmax(s, br, sim_ps, ncol, e_dst):
                rowmax = cols.tile([128, 1], f32, name=f"rm{s}{br}", tag="c1")
                nc.vector.tensor_reduce(out=rowmax[:], in_=sim_ps[:], axis=X,
                                        op=Alu.max)
                rmt = ps_ty.tile([1, 128], f32, name=f"rmt{s}{br}", tag="ty")
                nc.tensor.transpose(rmt[:], rowmax[:], identity[:])
                ngmax = cols.tile([1, 1], f32, name=f"ngm{s}{br}", tag="c0")
                nc.vector.tensor_reduce(out=ngmax[:], in_=rmt[:], axis=X,
                                        op=Alu.max, negate=True)
                nm_ps = ps_ty.tile([128, 1], f32, name=f"nmp{s}{br}", tag="ty")
                nc.tensor.matmul(nm_ps[:], ones_r[:], ngmax[:])
                nmcol = cols.tile([128, 1], f32, name=f"nmc{s}{br}", tag="c1")
                psum_copy_to(nmcol[:], nm_ps[:])
                rowsum = cols.tile([128, 1], f32, name=f"rs{s}{br}", tag="c1")
                nc.scalar.activation(out=e_dst[:], in_=sim_ps[:], func=Act.Exp,
                                     bias=nmcol[:], scale=1.0,
                                     accum_out=rowsum[:])
                tot_ps = ps_ty.tile([1, 1], f32, name=f"tot{s}{br}", tag="ty")
                nc.tensor.matmul(tot_ps[:], rowsum[:], ones_c[:])
                invt = keep.tile([1, 1], f32, name=f"it{s}{br}",
                                 tag=f"it{s}{br}")
                nc.vector.reciprocal(invt[:], tot_ps[:])
                ic_ps = ps_ty.tile([128, 1], f32, name=f"icp{s}{br}", tag="ty")
                nc.tensor.matmul(ic_ps[:], ones_r[:], invt[:])
                iccol = keep.tile([128, 1], f32, name=f"icc{s}{br}",
                                  tag=f"icc{s}{br}")
                psum_copy_to(iccol[:], ic_ps[:])
                it1[(s, br)] = invt
                itc[(s, br)] = iccol

            scht = bmap("scht", [128, CT * H], bf16)  # [h, (t,c_loc)]
            scwt = bmap("scwt", [128, CT * W], bf16)  # [w, (t,c_loc)]

            def emit_phaseB(s):
                transpose_to(scht[s], xch[s], CT, f"sch{s}")
                transpose_to(scwt[s], xcw[s], CT, f"scw{s}")

                # --- branch 12: sim12[w,c] = sum_h Shw[h,w] Sch[c,h]
                sim12 = ps_mm.tile([128, C], f32, name=f"s12_{s}", tag="mm")
                nc.tensor.matmul(sim12[:], shw[s][:], scht[s][:])
                softmax(s, 12, sim12, C, e12[s])
                transpose_to(e12t[s], e12[s], CT, f"e12{s}")
                # y12T[w,h] = sum_c e12t[c,w]^T ... accumulate 2 c tiles
                y12p = ps_mm.tile([128, H], f32, name=f"y12p{s}", tag="mm")
                for t in range(CT):
                    nc.tensor.matmul(
                        y12p[:], e12t[s][:, t * W:(t + 1) * W],
                        xch[s][:, t * H:(t + 1) * H],
                        start=(t == 0), stop=(t == CT - 1))
                psum_copy_to(y12T[s][:], y12p[:])
                # y21[c,h] per c tile
                for t in range(CT):
                    y21p = ps_mm.tile([128, H], f32, name=f"y21p{s}{t}",
                                      tag="mm")
                    nc.tensor.matmul(y21p[:], e12[s][:, t * 128:(t + 1) * 128],
                                     xhwT[s][:])
                    psum_copy_to(ymt(s, 0, t), y21p[:])

                # --- branch 13: sim13[h,c] = sum_w Shw[h,w] Scw[c,w]
                sim13 = ps_mm.tile([128, C], f32, name=f"s13_{s}", tag="mm")
                nc.tensor.matmul(sim13[:], xhwT[s][:], scwt[s][:])
                softmax(s, 13, sim13, C, e13[s])
                transpose_to(e13t[s], e13[s], CT, f"e13{s}")
                y13p = ps_mm.tile([128, W], f32, name=f"y13p{s}", tag="mm")
                for t in range(CT):
                    nc.tensor.matmul(
                        y13p[:], e13t[s][:, t * H:(t + 1) * H],
                        xcw[s][:, t * W:(t + 1) * W],
                        start=(t == 0), stop=(t == CT - 1))
                psum_copy_to(y13[s][:], y13p[:])
                for t in range(CT):
                    y31p = ps_mm.tile([128, W], f32, name=f"y31p{s}{t}",
                                      tag="mm")
                    nc.tensor.matmul(y31p[:], e13[s][:, t * 128:(t + 1) * 128],
                                     shw[s][:])
                    psum_copy_to(ymt(s, 2, t), y31p[:])

                # --- branch 23: sim23[h,w] = sum_c Sch[c,h] Scw[c,w]
                sim23 = ps_mm.tile([128, W], f32, name=f"s23_{s}", tag="mm")
                for t in range(CT):
                    nc.tensor.matmul(
                        sim23[:], xch[s][:, t * H:(t + 1) * H],
                        xcw[s][:, t * W:(t + 1) * W],
                        start=(t == 0), stop=(t == CT - 1))
                softmax(s, 23, sim23, W, e23[s])
                transpose_to(e23t[s], e23[s], 1, f"e23{s}")
                for t in range(CT):
                    y23p = ps_mm.tile([128, H], f32, name=f"y23p{s}{t}",
                                      tag="mm")
                    nc.tensor.matmul(y23p[:], scwt[s][:, t * W:(t + 1) * W],
                                     e23t[s][:])
                    psum_copy_to(ymt(s, 1, t), y23p[:])
                    y32p = ps_mm.tile([128, W], f32, name=f"y32p{s}{t}",
                                      tag="mm")
                    nc.tensor.matmul(y32p[:], scht[s][:, t * H:(t + 1) * H],
                                     e23[s][:])
                    psum_copy_to(ymt(s, 3, t), y32p[:])

                # y12 = transpose(y12T)  (f32)
                y12pp = ps_mm.tile([128, 128], f32, name=f"y12pp{s}", tag="mm")
                nc.tensor.transpose(y12pp[:], y12T[s][:], identity[:])
                psum_copy_to(y12[s][:], y12pp[:])

            # ---------------- phase C: BN partials ----------------
            ysq = gscr.tile([128, 128], f32, name="ysq", tag="ysq", bufs=2)

            def emit_phaseC(s):
                bnp = bnpS[s]
                it2 = {}
                for br in (12, 13, 23):
                    t2 = keep.tile([128, 1], f32, name=f"it2_{s}{br}",
                                   tag=f"it2_{s}{br}")
                    nc.vector.tensor_tensor(out=t2[:], in0=itc[(s, br)][:],
                                            in1=itc[(s, br)][:], op=Alu.mult)
                    it2[br] = t2
                bnc_maps = [(0, 12), (1, 23), (2, 13), (3, 23)]
                for m, br in bnc_maps:
                    r2 = cols.tile([128, 2], f32, name=f"r{s}{m}", tag="c2")
                    nc.vector.tensor_reduce(
                        out=r2[:], in_=ym(s, m).rearrange("p (t h) -> p t h",
                                                          t=CT),
                        axis=X, op=Alu.add)
                    nc.vector.scalar_tensor_tensor(
                        out=bnp[:, m * 2:m * 2 + 2], in0=r2[:],
                        scalar=itc[(s, br)][:], in1=bnp[:, m * 2:m * 2 + 2],
                        op0=Alu.mult, op1=Alu.add)
                    for t in range(CT):
                        col = m * 2 + t
                        blk = ymt(s, m, t)
                        sq = cols.tile([128, 1], f32, name=f"sq{s}{m}{t}",
                                       tag="c1")
                        nc.scalar.activation(out=ysq[:], in_=blk,
                                             func=Act.Square, accum_out=sq[:])
                        nc.vector.scalar_tensor_tensor(
                            out=bnp[:, 8 + col:9 + col], in0=sq[:],
                            scalar=it2[br][:], in1=bnp[:, 8 + col:9 + col],
                            op0=Alu.mult, op1=Alu.add)
                # bn1 partials (partition 0, cols 16..19)
                for j, (ymap, br) in enumerate(((y12T[s], 12), (y13[s], 13))):
                    i1 = it1[(s, br)]
                    i2 = cols.tile([1, 1], f32, name=f"i2_{s}{j}", tag="c0")
                    nc.vector.tensor_tensor(out=i2[:], in0=i1[:], in1=i1[:],
                                            op=Alu.mult)
                    rs = cols.tile([128, 1], f32, name=f"rs1_{s}{j}", tag="c1")
                    nc.vector.tensor_reduce(out=rs[:], in_=ymap[:], axis=X,
                                            op=Alu.add)
                    tp = ps_ty.tile([1, 1], f32, name=f"t1_{s}{j}", tag="ty")
                    nc.tensor.matmul(tp[:], rs[:], ones_c[:])
                    nc.vector.scalar_tensor_tensor(
                        out=bnp[0:1, 16 + 2 * j:17 + 2 * j], in0=tp[:],
                        scalar=i1[:], in1=bnp[0:1, 16 + 2 * j:17 + 2 * j],
                        op0=Alu.mult, op1=Alu.add)
                    sqc = cols.tile([128, 1], f32, name=f"sqc{s}{j}", tag="c1")
                    nc.scalar.activation(out=ysq[:], in_=ymap[:],
                                         func=Act.Square, accum_out=sqc[:])
                    tp2 = ps_ty.tile([1, 1], f32, name=f"t2_{s}{j}", tag="ty")
                    nc.tensor.matmul(tp2[:], sqc[:], ones_c[:])
                    nc.vector.scalar_tensor_tensor(
                        out=bnp[0:1, 17 + 2 * j:18 + 2 * j], in0=tp2[:],
                        scalar=i2[:], in1=bnp[0:1, 17 + 2 * j:18 + 2 * j],
                        op0=Alu.mult, op1=Alu.add)

            # interleaved emission: sample s's attention + BN partials
            # overlap the next sample's bulk pass-1 work in the in-order
            # engine queues
            for s in range(S):
                emit_pass1(s)
                if "B" in phases:
                    emit_phaseB(s)
                    if "C" in phases:
                        emit_phaseC(s)
            if "B" not in phases or "C" not in phases:
                return

            # pre-collective: per-sample itc columns in the (m,t) layout
            itc8 = []
            for s in range(S):
                t8 = keep.tile([128, 8], f32, name=f"itc8_{s}",
                               tag=f"itc8_{s}")
                for m, br in enumerate((12, 23, 13, 23)):
                    nc.vector.tensor_copy(
                        t8[:, 2 * m:2 * m + 2],
                        itc[(s, br)][:].broadcast_to([128, 2]))
                itc8.append(t8)

            # ---------------- allreduce ----------------
            nc.vector.tensor_tensor(out=bnp[:], in0=bnpS[0][:],
                                    in1=bnpS[1][:], op=Alu.add)
            nc.sync.dma_start(ccin, bnp[:])
            if n_cores > 1:
                nc.gpsimd.collective_compute(
                    "AllReduce", Alu.add,
                    replica_groups=[list(range(n_cores))],
                    ins=[ccin], outs=[ccout])
            else:
                nc.sync.dma_start(ccout, ccin)
            bnpg = persist.tile([128, NP], f32)
            nc.sync.dma_start(bnpg[:], ccout)

            # ---------------- phase D: BN finalize + gates ----------------
            # bn1 scalar chain first so the A gates (and the afull broadcast
            # DMA) unblock phase E as early as possible
            sc1 = []
            sh1 = []
            for j in range(2):
                mu1 = cols.tile([1, 1], f32, name=f"mu1_{j}", tag="c0")
                nc.vector.tensor_scalar_mul(
                    mu1[:], bnpg[0:1, 16 + 2 * j:17 + 2 * j], 1.0 / n1)
                m21 = cols.tile([1, 1], f32, name=f"m21_{j}", tag="c0")
                nc.vector.tensor_tensor(out=m21[:], in0=mu1[:], in1=mu1[:],
                                        op=Alu.mult)
                v1 = cols.tile([1, 1], f32, name=f"v1_{j}", tag="c0")
                nc.vector.scalar_tensor_tensor(
                    out=v1[:], in0=bnpg[0:1, 17 + 2 * j:18 + 2 * j],
                    scalar=1.0 / n1, in1=m21[:], op0=Alu.mult,
                    op1=Alu.subtract)
                sd1 = cols.tile([1, 1], f32, name=f"sd1_{j}", tag="c0")
                nc.scalar.activation(out=sd1[:], in_=v1[:], func=Act.Sqrt,
                                     bias=eps_col[0:1, :])
                rst1 = cols.tile([1, 1], f32, name=f"rst1_{j}", tag="c0")
                nc.vector.reciprocal(rst1[:], sd1[:])
                sc = keep.tile([1, 1], f32, name=f"sc1_{j}", tag=f"sc1_{j}")
                nc.vector.tensor_tensor(out=sc[:], in0=rst1[:],
                                        in1=bn1w_sb[:], op=Alu.mult)
                sc1.append(sc)
                q1 = cols.tile([1, 1], f32, name=f"q1_{j}", tag="c0")
                nc.vector.tensor_tensor(out=q1[:], in0=mu1[:], in1=sc[:],
                                        op=Alu.mult)
                sh = keep.tile([1, 1], f32, name=f"sh1_{j}", tag=f"sh1_{j}")
                nc.vector.scalar_tensor_tensor(
                    out=sh[:], in0=q1[:], scalar=-1.0, in1=bn1b_sb[:],
                    op0=Alu.mult, op1=Alu.add)
                sh1.append(sh)

            for s in range(S):
                rhs4 = cols.tile([1, 4], f32, name=f"rhs4_{s}", tag="c4")
                for j, br in ((0, 12), (1, 13)):
                    nc.vector.tensor_tensor(out=rhs4[:, j:j + 1],
                                            in0=sc1[j][:],
                                            in1=it1[(s, br)][:], op=Alu.mult)
                    nc.vector.tensor_copy(rhs4[:, 2 + j:3 + j], sh1[j][:])
                cm_ps = ps_ty.tile([128, 4], f32, name=f"cmp{s}", tag="ty")
                nc.tensor.matmul(cm_ps[:], ones_r[:], rhs4[:])
                colmat = cols.tile([128, 4], f32, name=f"cm{s}", tag="c4b")
                psum_copy_to(colmat[:], cm_ps[:])
                g1 = gscr.tile([128, W], bf16, name=f"g12_{s}", tag="ga")
                g2 = gscr.tile([128, W], bf16, name=f"g13_{s}", tag="ga")
                for j, (ymap, g) in enumerate(((y12[s], g1), (y13[s], g2))):
                    nc.scalar.activation(out=g[:], in_=ymap[:],
                                         func=Act.Sigmoid,
                                         bias=colmat[:, 2 + j:3 + j],
                                         scale=colmat[:, j:j + 1])
                nc.vector.tensor_tensor(out=agate[s][:], in0=g1[:], in1=g2[:],
                                        op=Alu.mult)
                nc.scalar.dma_start(
                    adram[s].rearrange("(h w) -> h w", h=H), agate[s][:])

            # bnc finalize (batched over all 4 maps x 2 c-tiles)
            sm = persist.tile([128, 8], f32, name="mu8")
            nc.vector.tensor_scalar_mul(sm[:], bnpg[:, 0:8], 1.0 / ncn)
            m2 = persist.tile([128, 8], f32, name="m28")
            nc.scalar.activation(out=m2[:], in_=bnpg[:, 0:8], func=Act.Square,
                                 scale=1.0 / ncn)
            var8 = persist.tile([128, 8], f32, name="var8")
            nc.vector.scalar_tensor_tensor(
                out=var8[:], in0=bnpg[:, 8:16], scalar=1.0 / ncn, in1=m2[:],
                op0=Alu.mult, op1=Alu.subtract)
            sd8 = persist.tile([128, 8], f32, name="sd8")
            nc.scalar.activation(out=sd8[:], in_=var8[:], func=Act.Sqrt,
                                 bias=eps_col[:])
            rstd8 = persist.tile([128, 8], f32, name="rstd8")
            nc.vector.reciprocal(rstd8[:], sd8[:])
            scale8 = persist.tile([128, 8], f32, name="scale8")
            nc.vector.tensor_tensor(out=scale8[:], in0=rstd8[:], in1=wc8[:],
                                    op=Alu.mult)
            q8 = persist.tile([128, 8], f32, name="q8")
            nc.vector.tensor_tensor(out=q8[:], in0=sm[:], in1=scale8[:],
                                    op=Alu.mult)
            shift8 = persist.tile([128, 8], f32, name="shift8")
            nc.vector.scalar_tensor_tensor(
                out=shift8[:], in0=q8[:], scalar=-1.0, in1=bc8[:],
                op0=Alu.mult, op1=Alu.add)

            bgate = smap("bgate", [128, CT * H], dtype=bf16)
            cgate = smap("cgate", [128, CT * W], dtype=bf16)
            for s in range(S):
                scc8 = cols.tile([128, 8], f32, name=f"scc8_{s}", tag="c8")
                nc.vector.tensor_tensor(out=scc8[:], in0=scale8[:],
                                        in1=itc8[s][:], op=Alu.mult)
                arg = gscr.tile([128, 8, 128], f32, name=f"arg{s}", tag="arg",
                                bufs=1)
                nc.vector.tensor_tensor(
                    out=arg[:],
                    in0=Y8[s][:].rearrange("p (m h) -> p m h", m=8),
                    in1=scc8[:].unsqueeze(2).broadcast_to([128, 8, 128]),
                    op=Alu.mult)
                nc.vector.tensor_tensor(
                    out=arg[:], in0=arg[:],
                    in1=shift8[:].unsqueeze(2).broadcast_to([128, 8, 128]),
                    op=Alu.add)
                gh = gscr.tile([128, 8, 128], bf16, name=f"gh{s}", tag="gh",
                               bufs=1)
                nc.scalar.activation(out=gh[:], in_=arg[:], func=Act.Sigmoid)
                nc.vector.tensor_tensor(
                    out=bgate[s][:].rearrange("p (t h) -> p t h", t=CT),
                    in0=gh[:, 0:2, :], in1=gh[:, 2:4, :], op=Alu.mult)
                nc.vector.tensor_tensor(
                    out=cgate[s][:].rearrange("p (t w) -> p t w", t=CT),
                    in0=gh[:, 4:6, :], in1=gh[:, 6:8, :], op=Alu.mult)

        # ---------------- phase E: apply (partition = channel) ----------------
        # out = (1 + A*B*C) * x, per chunk:
        #   mCA = afk * C[mid-bcast]          (DVE 2x)
        #   bexp = B[inner-bcast]             (Act copy)
        #   m = mCA * bexp                    (DVE 2x)
        #   m += 1                            (DVE 4x tensor_scalar)
        #   o = m * xres                      (DVE 2x)
        if "E" not in phases:
            return
        e_stack = contextlib.ExitStack()
        with e_stack:
            pme = e_stack.enter_context(tc.tile_pool(name="pme", bufs=2))
            paf = e_stack.enter_context(tc.tile_pool(name="paf", bufs=1))

            for s in range(S):
                # replicate A = g12*g13 [h,w] across the 128 c partitions
                # via stride-0 partition-broadcast DMA loads, one per h-chunk;
                # k-outer order so each A chunk serves both c tiles and the
                # replicas rotate through a single 3-buffer tag
                for k in range(NCH):
                    af = paf.tile([128, HC, W], bf16, name=f"af{s}{k}",
                                  tag="af", bufs=2)
                    nc.sync.dma_start(
                        af[:], adram[s].rearrange("(h w) -> h w", h=H)
                        [k * HC:(k + 1) * HC, :]
                        .unsqueeze(0).broadcast_to([128, HC, W]))
                    for t in range(CT):
                        csl = cgate[s][:, t * W:(t + 1) * W] \
                            .unsqueeze(1).broadcast_to([128, HC, W])
                        xsrc = xres[s][t][:, k * HC:(k + 1) * HC, :]
                        mca = pme.tile([128, HC, W], bf16,
                                       name=f"mca{s}{t}{k}", tag="mca")
                        nc.vector.tensor_tensor(out=mca[:], in0=af[:],
                                                in1=csl, op=Alu.mult)
                        bexp = pme.tile([128, HC, W], bf16,
                                        name=f"be{s}{t}{k}", tag="bexp")
                        bsl = bgate[s][:, t * H + k * HC:
                                       t * H + (k + 1) * HC] \
                            .unsqueeze(2).broadcast_to([128, HC, W])
                        if k % 2:
                            nc.gpsimd.tensor_copy(bexp[:], bsl)
                        else:
                            nc.scalar.copy(bexp[:], bsl)
                        m = pme.tile([128, HC, W], bf16, name=f"m{s}{t}{k}",
                                     tag="m")
                        nc.vector.tensor_tensor(out=m[:], in0=mca[:],
                                                in1=bexp[:], op=Alu.mult)
                        mp = pme.tile([128, HC, W], bf16, name=f"mp{s}{t}{k}",
                                      tag="mp")
                        if k % 2:
                            nc.vector.tensor_scalar_add(mp[:], m[:], 1.0)
                        else:
                            # m+1 on Act: Copy(1.0*m + 1.0)
                            nc.scalar.activation(out=mp[:], in_=m[:],
                                                 func=Act.Copy, bias=1.0)
                        o = pme.tile([128, HC, W], bf16, name=f"oe{s}{t}{k}",
                                     tag="oe")
                        nc.vector.tensor_tensor(out=o[:], in0=mp[:], in1=xsrc,
                                                op=Alu.mult)
                        nc.scalar.dma_start(
                            outy[s, t * 128:(t + 1) * 128,
                                 k * HC:(k + 1) * HC, :], o[:])


_NC_CACHE = {}
LAST_RESULT = None


def _get_nc(n_cores: int, sync_start: bool = False, phases: str = "ABCDE"):
    key = (n_cores, sync_start, phases)
    if key not in _NC_CACHE:
        _NC_CACHE[key] = build_bass(n_cores, sync_start, phases)
    return _NC_CACHE[key]


def kernel(**inputs) -> np.ndarray:
    from concourse.bass_utils import run_bass_kernel_spmd

    x = np.ascontiguousarray(inputs["x"], dtype=np.float32)
    bn1_w = np.ascontiguousarray(inputs["bn1_w"], dtype=np.float32)
    bn1_b = np.ascontiguousarray(inputs["bn1_b"], dtype=np.float32)
    bnc_w = np.ascontiguousarray(inputs["bnc_w"], dtype=np.float32)
    bnc_b = np.ascontiguousarray(inputs["bnc_b"], dtype=np.float32)
    B = x.shape[0]
    assert B == NCORES * S, (B, NCORES, S)

    nc = _get_nc(NCORES)
    in_maps = []
    for i in range(NCORES):
        in_maps.append({
            "xs": np.ascontiguousarray(x[i * S:(i + 1) * S]),
            "bn1_w": bn1_w, "bn1_b": bn1_b,
            "bnc_w": bnc_w, "bnc_b": bnc_b,
        })
    res = run_bass_kernel_spmd(nc, in_maps, core_ids=list(range(NCORES)))
    global LAST_RESULT
    LAST_RESULT = res
    out = np.concatenate(
        [np.asarray(res.results[i]["outy"]).astype(np.float32)
         for i in range(NCORES)], axis=0)
    return out


# revision 14
# speedup vs baseline: 11.2486x; 11.2486x over previous
"""Trainium2 Bass kernel for nn_Cross_Attention (triplet-pool cross-attention gating).

Math (per sample b):
  pools:  Shw[h,w]=max_c x,  Sch[c,h]=max_w x,  Scw[c,w]=max_h x
  3 branches of flat-softmax cross attention between pools -> y12,y13 [h,w],
  y21,y23 [c,h], y31,y32 [c,w]
  training-mode BatchNorm over the *global* batch (cross-core allreduce of
  sum/sumsq), sigmoid gates, and finally
  out = x * (g12*g13)[h,w] * (g21*g23)[c,h] * (g31*g32)[c,w] + x
      = x * (1 + A[h,w]*B[c,h]*Cg[c,w])

Sharding: batch-parallel, 2 samples per core on 8 cores; only the BN batch
stats cross cores (AllReduce of a [128,20] tile).

Engine split (v2): x is converted to a resident bf16 copy on the Act engine;
all three pools come from that copy -- Sch/Scw via DVE tensor_tensor max
trees (bf16 packed operands run the DVE at 2x), Shw via a gpsimd pair-max +
cross-partition tensor_reduce (axis=C) with a small SBUF->SBUF DMA to lay the
rows out partition=h.  The apply phase is three 2x TT ops plus a 4x
tensor-scalar (+1): m = (A*C)[mid-bcast] * B_exp, out = (m+1)*x, where B_exp
is materialised by the Act engine (innermost-stride-0 operands would drop the
DVE to 1x).
"""

import numpy as np

import concourse.bacc as bacc
import concourse.mybir as mybir
import concourse.tile as tile
from concourse import masks
import concourse.bass_isa as bass_isa

f32 = mybir.dt.float32
bf16 = mybir.dt.bfloat16
Alu = mybir.AluOpType
Act = mybir.ActivationFunctionType
X = mybir.AxisListType.X
CAX = mybir.AxisListType.C

NCORES = 8
S = 2          # samples per core
C, H, W = 256, 128, 128
CT = 2         # c tiles of 128
HC = 16        # h rows per chunk
NCH = H // HC  # 8
NP = 20        # bnp columns
EPS = 1e-5


def build_bass(n_cores: int, sync_start: bool = False, phases: str = "ABCDE"):
    """sync_start/phases are for timing probes only: sync_start prepends a
    tiny AllReduce so all cores start main work in lockstep (makes full
    device time visible to the marginal-time harness); phases truncates."""
    nc = bacc.Bacc("TRN2", target_bir_lowering=False, debug=False,
                   num_devices=n_cores)
    nb_tot = n_cores * S
    n1 = float(nb_tot * H * W)   # bn1 count
    ncn = float(nb_tot * H)      # bnc count (per channel)

    xs = nc.dram_tensor("xs", [S, C, H, W], f32, kind="ExternalInput").ap()
    bn1w = nc.dram_tensor("bn1_w", [1], f32, kind="ExternalInput").ap()
    bn1b = nc.dram_tensor("bn1_b", [1], f32, kind="ExternalInput").ap()
    bncw = nc.dram_tensor("bnc_w", [C], f32, kind="ExternalInput").ap()
    bncb = nc.dram_tensor("bnc_b", [C], f32, kind="ExternalInput").ap()
    outy = nc.dram_tensor("outy", [S, C, H, W], bf16,
                          kind="ExternalOutput").ap()

    ccin = nc.dram_tensor("ccin", [128, NP], f32).ap()
    ccout = nc.dram_tensor(
        "ccout", [128, NP], f32,
        addr_space="Shared" if n_cores > 1 else "Local").ap()
    adram = nc.dram_tensor("adram", [S, H * W], bf16).ap()
    srow = nc.dram_tensor("srow", [S, H * W], bf16).ap()
    sync_bufs = None
    if sync_start:
        sin = nc.dram_tensor("sin", [1, 1], f32).ap()
        sout = nc.dram_tensor(
            "sout", [1, 1], f32,
            addr_space="Shared" if n_cores > 1 else "Local").ap()
        sync_bufs = (sin, sout)

    with tile.TileContext(nc) as tc:
        _emit(nc, tc, n_cores, n1, ncn,
              xs, bn1w, bn1b, bncw, bncb, outy, ccin, ccout, adram, srow,
              sync_bufs, phases)
    nc.compile()
    return nc


def _emit(nc, tc, n_cores, n1, ncn,
          xs, bn1w, bn1b, bncw, bncb, outy, ccin, ccout, adram, srow,
          sync_bufs=None, phases="ABCDE"):
    import contextlib
    stack = contextlib.ExitStack()
    with stack:
        persist = stack.enter_context(tc.tile_pool(name="persist", bufs=1))
        maps = stack.enter_context(tc.tile_pool(name="maps", bufs=2))
        cols = stack.enter_context(tc.tile_pool(name="cols", bufs=4))
        keep = stack.enter_context(tc.tile_pool(name="keep", bufs=1))
        gscr = stack.enter_context(tc.tile_pool(name="gscr", bufs=4))

        # --- timing-only start barrier: a tiny AllReduce whose result is
        # loaded on the sync DMA queue, so every later HWDGE load (FIFO per
        # engine) waits until all cores have started this iteration ---
        if sync_bufs is not None:
            sin, sout = sync_bufs
            st0 = cols.tile([1, 1], f32, name="st0", tag="c0")
            nc.vector.memset(st0[:], 1.0)
            nc.sync.dma_start(sin, st0[:])
            nc.gpsimd.collective_compute(
                "AllReduce", Alu.add,
                replica_groups=[list(range(n_cores))],
                ins=[sin], outs=[sout])
            st1 = cols.tile([1, 1], f32, name="st1", tag="c0")
            nc.sync.dma_start(st1[:], sout)
        else:
            st1 = None

        # --- setup ---
        identity = persist.tile([128, 128], f32)
        masks.make_identity(nc, identity[:])
        identb = persist.tile([128, 128], bf16)
        nc.vector.tensor_copy(identb[:], identity[:])
        ones_r = persist.tile([1, 128], f32)
        nc.vector.memset(ones_r[:], 1.0)
        ones_c = persist.tile([128, 1], f32)
        nc.vector.memset(ones_c[:], 1.0)
        eps_col = persist.tile([128, 1], f32)
        nc.vector.memset(eps_col[:], EPS)
        wc2 = persist.tile([128, 2], f32)
        nc.sync.dma_start(wc2[:], bncw.rearrange("(t c) -> c t", c=128))
        bc2 = persist.tile([128, 2], f32)
        nc.sync.dma_start(bc2[:], bncb.rearrange("(t c) -> c t", c=128))
        bn1w_sb = persist.tile([1, 1], f32)
        nc.sync.dma_start(bn1w_sb[:], bn1w.unsqueeze(1))
        bn1b_sb = persist.tile([1, 1], f32)
        nc.sync.dma_start(bn1b_sb[:], bn1b.unsqueeze(1))
        wc8 = persist.tile([128, 8], f32)
        bc8 = persist.tile([128, 8], f32)
        for m in range(4):
            nc.vector.tensor_copy(wc8[:, m * 2:m * 2 + 2], wc2[:])
            nc.vector.tensor_copy(bc8[:, m * 2:m * 2 + 2], bc2[:])
        bnpS = []
        for s in range(S):
            t_ = persist.tile([128, NP], f32, name=f"bnpS{s}")
            nc.vector.memset(t_[:], 0.0)
            bnpS.append(t_)
        bnp = persist.tile([128, NP], f32)

        # per-sample persistent maps (bufs=2 -> one slot per sample)
        def smap(name, shape, bufs=None, dtype=f32):
            return [maps.tile(shape, dtype, name=f"{name}{s}", tag=name,
                              bufs=bufs)
                    for s in range(S)]

        xch = smap("xch", [128, CT * H], dtype=bf16)    # [c_loc, (t,h)]
        xcw = smap("xcw", [128, CT * W], dtype=bf16)    # [c_loc, (t,w)]
        xhwT = smap("xhwT", [128, H], dtype=bf16)       # [w, h]
        shw = smap("shw", [128, W], dtype=bf16)         # [h, w]
        y12 = smap("y12", [128, W])         # [h, w] f32
        y13 = smap("y13", [128, W])         # [h, w] f32
        # the four bnc y-maps live in one [128, (m,t)*128] tile per sample,
        # cols (m, t): m0=y21(br12) m1=y23(br23) m2=y31(br13) m3=y32(br23),
        # so the whole B/C gate batch is one sigmoid activation later
        Y8 = smap("Y8", [128, 8 * 128])

        def ymt(s, m, t):
            return Y8[s][:, (2 * m + t) * 128:(2 * m + t + 1) * 128]

        def ym(s, m):
            return Y8[s][:, 2 * m * 128:(2 * m + 2) * 128]
        agate = smap("agate", [128, W], dtype=bf16)     # [h, w]
        itc = {}   # invT cols [128,1] per (s, branch)
        it1 = {}   # invT [1,1] per (s, branch)

        # resident bf16 x: [c_loc, h, w] per (s, t) -- conversion on Act,
        # source for all pools and the apply phase
        xres = [[persist.tile([128, H, W], bf16,
                              name=f"xres{s}{t}", tag=f"xres{s}{t}")
                 for t in range(CT)] for s in range(S)]

        ps_stack = contextlib.ExitStack()
        with ps_stack:
            ptree = ps_stack.enter_context(tc.tile_pool(name="ptree", bufs=1))
            ppm = ps_stack.enter_context(tc.tile_pool(name="ppm", bufs=2))
            pbm = ps_stack.enter_context(tc.tile_pool(name="pbm", bufs=1))

            def bmap(name, shape, dtype=f32):
                return [pbm.tile(shape, dtype, name=f"{name}{s}", tag=name)
                        for s in range(S)]

            e12 = bmap("e12", [128, C], bf16)         # [w, c]
            e12t = bmap("e12t", [128, CT * W], bf16)  # [c_loc, (t,w)]
            e13 = bmap("e13", [128, C], bf16)         # [h, c]
            e13t = bmap("e13t", [128, CT * H], bf16)  # [c_loc, (t,h)]
            e23 = bmap("e23", [128, W], bf16)         # [h, w]
            e23t = bmap("e23t", [128, H], bf16)       # [w, h]
            y12T = bmap("y12T", [128, H])             # [w, h] f32
            ps_mm = ps_stack.enter_context(
                tc.tile_pool(name="ps_mm", bufs=2, space="PSUM"))
            ps_ty = ps_stack.enter_context(
                tc.tile_pool(name="ps_ty", bufs=2, space="PSUM"))

            # ---------------- pass 1: load, convert, pools ----------------
            def emit_pass1(s):
                HH = H // 2
                for half in range(2):
                    for t in range(CT):
                        # casting DMA (gpsimd-initiated SWDGE): f32 DRAM ->
                        # bf16 resident copy, half a c-tile per trigger to
                        # amortise the SWDGE fixed cost on the Pool queue
                        dst = xres[s][t][:, half * HH:(half + 1) * HH, :]
                        if st1 is not None:
                            # timing probe: force every load to wait for the
                            # start barrier via a WAW dep on the tile
                            nc.vector.tensor_copy(
                                xres[s][t][0:1, half * HH, 0:1], st1[:])
                        nc.gpsimd.dma_start(
                            dst, xs[s, t * 128:(t + 1) * 128,
                                    half * HH:(half + 1) * HH, :])
                    for k in range(half * (NCH // 2), (half + 1) * (NCH // 2)):
                        # Shw: pair-max over the two c tiles (alternating
                        # DVE/Pool), then max over the 128 c_loc partitions
                        # (Pool, axis=C), then a DRAM bounce to partition=h
                        pm = ppm.tile([128, HC, W], bf16, name=f"pm{s}{k}",
                                      tag="pm")
                        # neuronxcc rejects TensorTensor on Pool; DVE 2x
                        nc.vector.tensor_tensor(
                            out=pm[:],
                            in0=xres[s][0][:, k * HC:(k + 1) * HC, :],
                            in1=xres[s][1][:, k * HC:(k + 1) * HC, :],
                            op=Alu.max)
                        row = ppm.tile([128, HC * W], bf16,
                                       name=f"row{s}{k}", tag="row")
                        nc.gpsimd.partition_all_reduce(
                            row[:], pm[:].rearrange("p h w -> p (h w)"),
                            channels=128, reduce_op=bass_isa.ReduceOp.max)
                        nc.sync.dma_start(
                            srow[s, k * HC * W:(k + 1) * HC * W].unsqueeze(0),
                            row[0:1, :])
                # Shw rows land partition=h via a DRAM bounce
                nc.sync.dma_start(
                    shw[s][:], srow[s].rearrange("(h w) -> h w", h=H))

                # Sch / Scw via DVE bf16 max trees (2x packed mode).
                # Quarter-fold first so every scratch tile is <= 8KB and all
                # levels cycle through one shared 3-buffer tag.
                def tscr(shape, nm):
                    return ptree.tile(shape, bf16, name=nm, tag="tr", bufs=3)

                for t in range(CT):
                    xr = xres[s][t]
                    # Scw: fold h 128 -> 1
                    q1 = tscr([128, 32, W], f"cwq1{s}{t}")
                    nc.vector.tensor_tensor(out=q1[:], in0=xr[:, 0:32, :],
                                            in1=xr[:, 64:96, :], op=Alu.max)
                    q2 = tscr([128, 32, W], f"cwq2{s}{t}")
                    nc.vector.tensor_tensor(out=q2[:], in0=xr[:, 32:64, :],
                                            in1=xr[:, 96:128, :], op=Alu.max)
                    cur = tscr([128, 32, W], f"cwm32{s}{t}")
                    nc.vector.tensor_tensor(out=cur[:], in0=q1[:], in1=q2[:],
                                            op=Alu.max)
                    hs = 16
                    while hs >= 1:
                        if hs == 1:
                            dst_ap = xcw[s][:, t * W:(t + 1) * W].unsqueeze(1)
                        else:
                            nt = tscr([128, hs, W], f"cw{s}{t}{hs}")
                            dst_ap = nt[:]
                        nc.vector.tensor_tensor(
                            out=dst_ap, in0=cur[:, 0:hs, :],
                            in1=cur[:, hs:2 * hs, :], op=Alu.max)
                        if hs > 1:
                            cur = nt
                        hs //= 2
                    # Sch: fold w 128 -> 1
                    q1 = tscr([128, H, 32], f"chq1{s}{t}")
                    nc.vector.tensor_tensor(out=q1[:], in0=xr[:, :, 0:32],
                                            in1=xr[:, :, 64:96], op=Alu.max)
                    q2 = tscr([128, H, 32], f"chq2{s}{t}")
                    nc.vector.tensor_tensor(out=q2[:], in0=xr[:, :, 32:64],
                                            in1=xr[:, :, 96:128], op=Alu.max)
                    cur = tscr([128, H, 32], f"chm32{s}{t}")
                    nc.vector.tensor_tensor(out=cur[:], in0=q1[:], in1=q2[:],
                                            op=Alu.max)
                    ws = 16
                    while ws >= 1:
                        if ws == 1:
                            dst_ap = xch[s][:, t * H:(t + 1) * H].unsqueeze(2)
                        else:
                            nt = tscr([128, H, ws], f"ch{s}{t}{ws}")
                            dst_ap = nt[:]
                        nc.vector.tensor_tensor(
                            out=dst_ap, in0=cur[:, :, 0:ws],
                            in1=cur[:, :, ws:2 * ws], op=Alu.max)
                        if ws > 1:
                            cur = nt
                        ws //= 2

                # xhwT = transpose(shw)
                tp = ps_ty.tile([128, 128], bf16, name=f"shwT{s}", tag="tyb")
                nc.tensor.transpose(tp[:], shw[s][:], identb[:])
                nc.vector.tensor_copy(xhwT[s][:], tp[:])

            # ---------------- phase B: attention ----------------
            def psum_copy_to(dst, src_ps):
                nc.scalar.copy(dst, src_ps)

            def transpose_to(dst, src_sb, nblk, name):
                # src [128, nblk*128] bf16 -> dst [128, nblk*128] blockwise T
                for t in range(nblk):
                    tp = ps_mm.tile([128, 128], bf16, name=f"tp{name}{t}",
                                    tag="mmb")
                    nc.tensor.transpose(
                        tp[:], src_sb[:, t * 128:(t + 1) * 128], identb[:])
                    psum_copy_to(dst[:, t * 128:(t + 1) * 128], tp[:])

            def softmax(s, br, sim_ps, ncol, e_dst):
                rowmax = cols.tile([128, 1], f32, name=f"rm{s}{br}", tag="c1")
                nc.vector.tensor_reduce(out=rowmax[:], in_=sim_ps[:], axis=X,
                                        op=Alu.max)
                rmt = ps_ty.tile([1, 128], f32, name=f"rmt{s}{br}", tag="ty")
                nc.tensor.transpose(rmt[:], rowmax[:], identity[:])
                ngmax = cols.tile([1, 1], f32, name=f"ngm{s}{br}", tag="c0")
                nc.vector.tensor_reduce(out=ngmax[:], in_=rmt[:], axis=X,
                                        op=Alu.max, negate=True)
                nm_ps = ps_ty.tile([128, 1], f32, name=f"nmp{s}{br}", tag="ty")
                nc.tensor.matmul(nm_ps[:], ones_r[:], ngmax[:])
                nmcol = cols.tile([128, 1], f32, name=f"nmc{s}{br}", tag="c1")
                psum_copy_to(nmcol[:], nm_ps[:])
                rowsum = cols.tile([128, 1], f32, name=f"rs{s}{br}", tag="c1")
                nc.scalar.activation(out=e_dst[:], in_=sim_ps[:], func=Act.Exp,
                                     bias=nmcol[:], scale=1.0,
                                     accum_out=rowsum[:])
                tot_ps = ps_ty.tile([1, 1], f32, name=f"tot{s}{br}", tag="ty")
                nc.tensor.matmul(tot_ps[:], rowsum[:], ones_c[:])
                invt = keep.tile([1, 1], f32, name=f"it{s}{br}",
                                 tag=f"it{s}{br}")
                nc.vector.reciprocal(invt[:], tot_ps[:])
                ic_ps = ps_ty.tile([128, 1], f32, name=f"icp{s}{br}", tag="ty")
                nc.tensor.matmul(ic_ps[:], ones_r[:], invt[:])
                iccol = keep.tile([128, 1], f32, name=f"icc{s}{br}",
                                  tag=f"icc{s}{br}")
                psum_copy_to(iccol[:], ic_ps[:])
                it1[(s, br)] = invt
                itc[(s, br)] = iccol

            scht = bmap("scht", [128, CT * H], bf16)  # [h, (t,c_loc)]
            scwt = bmap("scwt", [128, CT * W], bf16)  # [w, (t,c_loc)]

            def emit_phaseB(s):
                transpose_to(scht[s], xch[s], CT, f"sch{s}")
                transpose_to(scwt[s], xcw[s], CT, f"scw{s}")

                # --- branch 12: sim12[w,c] = sum_h Shw[h,w] Sch[c,h]
                sim12 = ps_mm.tile([128, C], f32, name=f"s12_{s}", tag="mm")
                nc.tensor.matmul(sim12[:], shw[s][:], scht[s][:])
                softmax(s, 12, sim12, C, e12[s])
                transpose_to(e12t[s], e12[s], CT, f"e12{s}")
                # y12T[w,h] = sum_c e12t[c,w]^T ... accumulate 2 c tiles
                y12p = ps_mm.tile([128, H], f32, name=f"y12p{s}", tag="mm")
                for t in range(CT):
                    nc.tensor.matmul(
                        y12p[:], e12t[s][:, t * W:(t + 1) * W],
                        xch[s][:, t * H:(t + 1) * H],
                        start=(t == 0), stop=(t == CT - 1))
                psum_copy_to(y12T[s][:], y12p[:])
                # y21[c,h] per c tile
                for t in range(CT):
                    y21p = ps_mm.tile([128, H], f32, name=f"y21p{s}{t}",
                                      tag="mm")
                    nc.tensor.matmul(y21p[:], e12[s][:, t * 128:(t + 1) * 128],
                                     xhwT[s][:])
                    psum_copy_to(ymt(s, 0, t), y21p[:])

                # --- branch 13: sim13[h,c] = sum_w Shw[h,w] Scw[c,w]
                sim13 = ps_mm.tile([128, C], f32, name=f"s13_{s}", tag="mm")
                nc.tensor.matmul(sim13[:], xhwT[s][:], scwt[s][:])
                softmax(s, 13, sim13, C, e13[s])
                transpose_to(e13t[s], e13[s], CT, f"e13{s}")
                y13p = ps_mm.tile([128, W], f32, name=f"y13p{s}", tag="mm")
                for t in range(CT):
                    nc.tensor.matmul(
                        y13p[:], e13t[s][:, t * H:(t + 1) * H],
                        xcw[s][:, t * W:(t + 1) * W],
                        start=(t == 0), stop=(t == CT - 1))
                psum_copy_to(y13[s][:], y13p[:])
                for t in range(CT):
                    y31p = ps_mm.tile([128, W], f32, name=f"y31p{s}{t}",
                                      tag="mm")
                    nc.tensor.matmul(y31p[:], e13[s][:, t * 128:(t + 1) * 128],
                                     shw[s][:])
                    psum_copy_to(ymt(s, 2, t), y31p[:])

                # --- branch 23: sim23[h,w] = sum_c Sch[c,h] Scw[c,w]
                sim23 = ps_mm.tile([128, W], f32, name=f"s23_{s}", tag="mm")
                for t in range(CT):
                    nc.tensor.matmul(
                        sim23[:], xch[s][:, t * H:(t + 1) * H],
                        xcw[s][:, t * W:(t + 1) * W],
                        start=(t == 0), stop=(t == CT - 1))
                softmax(s, 23, sim23, W, e23[s])
                transpose_to(e23t[s], e23[s], 1, f"e23{s}")
                for t in range(CT):
                    y23p = ps_mm.tile([128, H], f32, name=f"y23p{s}{t}",
                                      tag="mm")
                    nc.tensor.matmul(y23p[:], scwt[s][:, t * W:(t + 1) * W],
                                     e23t[s][:])
                    psum_copy_to(ymt(s, 1, t), y23p[:])
                    y32p = ps_mm.tile([128, W], f32, name=f"y32p{s}{t}",
                                      tag="mm")
                    nc.tensor.matmul(y32p[:], scht[s][:, t * H:(t + 1) * H],
                                     e23[s][:])
                    psum_copy_to(ymt(s, 3, t), y32p[:])

                # y12 = transpose(y12T)  (f32)
                y12pp = ps_mm.tile([128, 128], f32, name=f"y12pp{s}", tag="mm")
                nc.tensor.transpose(y12pp[:], y12T[s][:], identity[:])
                psum_copy_to(y12[s][:], y12pp[:])

            # ---------------- phase C: BN partials ----------------
            ysq = gscr.tile([128, 128], f32, name="ysq", tag="ysq", bufs=2)

            def emit_phaseC(s):
                bnp = bnpS[s]
                it2 = {}
                for br in (12, 13, 23):
                    t2 = keep.tile([128, 1], f32, name=f"it2_{s}{br}",
                                   tag=f"it2_{s}{br}")
                    nc.vector.tensor_tensor(out=t2[:], in0=itc[(s, br)][:],
                                            in1=itc[(s, br)][:], op=Alu.mult)
                    it2[br] = t2
                bnc_maps = [(0, 12), (1, 23), (2, 13), (3, 23)]
                for m, br in bnc_maps:
                    r2 = cols.tile([128, 2], f32, name=f"r{s}{m}", tag="c2")
                    nc.vector.tensor_reduce(
                        out=r2[:], in_=ym(s, m).rearrange("p (t h) -> p t h",
                                                          t=CT),
                        axis=X, op=Alu.add)
                    nc.vector.scalar_tensor_tensor(
                        out=bnp[:, m * 2:m * 2 + 2], in0=r2[:],
                        scalar=itc[(s, br)][:], in1=bnp[:, m * 2:m * 2 + 2],
                        op0=Alu.mult, op1=Alu.add)
                    for t in range(CT):
                        col = m * 2 + t
                        blk = ymt(s, m, t)
                        sq = cols.tile([128, 1], f32, name=f"sq{s}{m}{t}",
                                       tag="c1")
                        nc.scalar.activation(out=ysq[:], in_=blk,
                                             func=Act.Square, accum_out=sq[:])
                        nc.vector.scalar_tensor_tensor(
                            out=bnp[:, 8 + col:9 + col], in0=sq[:],
                            scalar=it2[br][:], in1=bnp[:, 8 + col:9 + col],
                            op0=Alu.mult, op1=Alu.add)
                # bn1 partials (partition 0, cols 16..19)
                for j, (ymap, br) in enumerate(((y12T[s], 12), (y13[s], 13))):
                    i1 = it1[(s, br)]
                    i2 = cols.tile([1, 1], f32, name=f"i2_{s}{j}", tag="c0")
                    nc.vector.tensor_tensor(out=i2[:], in0=i1[:], in1=i1[:],
                                            op=Alu.mult)
                    rs = cols.tile([128, 1], f32, name=f"rs1_{s}{j}", tag="c1")
                    nc.vector.tensor_reduce(out=rs[:], in_=ymap[:], axis=X,
                                            op=Alu.add)
                    tp = ps_ty.tile([1, 1], f32, name=f"t1_{s}{j}", tag="ty")
                    nc.tensor.matmul(tp[:], rs[:], ones_c[:])
                    nc.vector.scalar_tensor_tensor(
                        out=bnp[0:1, 16 + 2 * j:17 + 2 * j], in0=tp[:],
                        scalar=i1[:], in1=bnp[0:1, 16 + 2 * j:17 + 2 * j],
                        op0=Alu.mult, op1=Alu.add)
                    sqc = cols.tile([128, 1], f32, name=f"sqc{s}{j}", tag="c1")
                    nc.scalar.activation(out=ysq[:], in_=ymap[:],
                                         func=Act.Square, accum_out=sqc[:])
                    tp2 = ps_ty.tile([1, 1], f32, name=f"t2_{s}{j}", tag="ty")
                    nc.tensor.matmul(tp2[:], sqc[:], ones_c[:])
                    nc.vector.scalar_tensor_tensor(
                        out=bnp[0:1, 17 + 2 * j:18 + 2 * j], in0=tp2[:],
                        scalar=i2[:], in1=bnp[0:1, 17 + 2 * j:18 + 2 * j],
                        op0=Alu.mult, op1=Alu.add)

            # interleaved emission: sample s's attention + BN partials
            # overlap the next sample's bulk pass-1 work in the in-order
            # engine queues
            for s in range(S):
                emit_pass1(s)
                if "B" in phases:
                    emit_phaseB(s)
                    if "C" in phases:
                        emit_phaseC(s)
            if "B" not in phases or "C" not in phases:
                return

            # pre-collective: per-sample itc columns in the (m,t) layout
            itc8 = []
            for s in range(S):
                t8 = keep.tile([128, 8], f32, name=f"itc8_{s}",
                               tag=f"itc8_{s}")
                for m, br in enumerate((12, 23, 13, 23)):
                    nc.vector.tensor_copy(
                        t8[:, 2 * m:2 * m + 2],
                        itc[(s, br)][:].broadcast_to([128, 2]))
                itc8.append(t8)

            # ---------------- allreduce ----------------
            nc.vector.tensor_tensor(out=bnp[:], in0=bnpS[0][:],
                                    in1=bnpS[1][:], op=Alu.add)
            nc.sync.dma_start(ccin, bnp[:])
            if n_cores > 1:
                nc.gpsimd.collective_compute(
                    "AllReduce", Alu.add,
                    replica_groups=[list(range(n_cores))],
                    ins=[ccin], outs=[ccout])
            else:
                nc.sync.dma_start(ccout, ccin)
            bnpg = persist.tile([128, NP], f32)
            nc.sync.dma_start(bnpg[:], ccout)

            # ---------------- phase D: BN finalize + gates ----------------
            # bn1 scalar chain first so the A gates (and the afull broadcast
            # DMA) unblock phase E as early as possible
            sc1 = []
            sh1 = []
            for j in range(2):
                mu1 = cols.tile([1, 1], f32, name=f"mu1_{j}", tag="c0")
                nc.vector.tensor_scalar_mul(
                    mu1[:], bnpg[0:1, 16 + 2 * j:17 + 2 * j], 1.0 / n1)
                m21 = cols.tile([1, 1], f32, name=f"m21_{j}", tag="c0")
                nc.vector.tensor_tensor(out=m21[:], in0=mu1[:], in1=mu1[:],
                                        op=Alu.mult)
                v1 = cols.tile([1, 1], f32, name=f"v1_{j}", tag="c0")
                nc.vector.scalar_tensor_tensor(
                    out=v1[:], in0=bnpg[0:1, 17 + 2 * j:18 + 2 * j],
                    scalar=1.0 / n1, in1=m21[:], op0=Alu.mult,
                    op1=Alu.subtract)
                sd1 = cols.tile([1, 1], f32, name=f"sd1_{j}", tag="c0")
                nc.scalar.activation(out=sd1[:], in_=v1[:], func=Act.Sqrt,
                                     bias=eps_col[0:1, :])
                rst1 = cols.tile([1, 1], f32, name=f"rst1_{j}", tag="c0")
                nc.vector.reciprocal(rst1[:], sd1[:])
                sc = keep.tile([1, 1], f32, name=f"sc1_{j}", tag=f"sc1_{j}")
                nc.vector.tensor_tensor(out=sc[:], in0=rst1[:],
                                        in1=bn1w_sb[:], op=Alu.mult)
                sc1.append(sc)
                q1 = cols.tile([1, 1], f32, name=f"q1_{j}", tag="c0")
                nc.vector.tensor_tensor(out=q1[:], in0=mu1[:], in1=sc[:],
                                        op=Alu.mult)
                sh = keep.tile([1, 1], f32, name=f"sh1_{j}", tag=f"sh1_{j}")
                nc.vector.scalar_tensor_tensor(
                    out=sh[:], in0=q1[:], scalar=-1.0, in1=bn1b_sb[:],
                    op0=Alu.mult, op1=Alu.add)
                sh1.append(sh)

            for s in range(S):
                rhs4 = cols.tile([1, 4], f32, name=f"rhs4_{s}", tag="c4")
                for j, br in ((0, 12), (1, 13)):
                    nc.vector.tensor_tensor(out=rhs4[:, j:j + 1],
                                            in0=sc1[j][:],
                                            in1=it1[(s, br)][:], op=Alu.mult)
                    nc.vector.tensor_copy(rhs4[:, 2 + j:3 + j], sh1[j][:])
                cm_ps = ps_ty.tile([128, 4], f32, name=f"cmp{s}", tag="ty")
                nc.tensor.matmul(cm_ps[:], ones_r[:], rhs4[:])
                colmat = cols.tile([128, 4], f32, name=f"cm{s}", tag="c4b")
                psum_copy_to(colmat[:], cm_ps[:])
                g1 = gscr.tile([128, W], bf16, name=f"g12_{s}", tag="ga")
                g2 = gscr.tile([128, W], bf16, name=f"g13_{s}", tag="ga")
                for j, (ymap, g) in enumerate(((y12[s], g1), (y13[s], g2))):
                    nc.scalar.activation(out=g[:], in_=ymap[:],
                                         func=Act.Sigmoid,
                                         bias=colmat[:, 2 + j:3 + j],
                                         scale=colmat[:, j:j + 1])
                nc.vector.tensor_tensor(out=agate[s][:], in0=g1[:], in1=g2[:],
                                        op=Alu.mult)
                nc.scalar.dma_start(
                    adram[s].rearrange("(h w) -> h w", h=H), agate[s][:])

            # bnc finalize (batched over all 4 maps x 2 c-tiles)
            sm = persist.tile([128, 8], f32, name="mu8")
            nc.vector.tensor_scalar_mul(sm[:], bnpg[:, 0:8], 1.0 / ncn)
            m2 = persist.tile([128, 8], f32, name="m28")
            nc.scalar.activation(out=m2[:], in_=bnpg[:, 0:8], func=Act.Square,
                                 scale=1.0 / ncn)
            var8 = persist.tile([128, 8], f32, name="var8")
            nc.vector.scalar_tensor_tensor(
                out=var8[:], in0=bnpg[:, 8:16], scalar=1.0 / ncn, in1=m2[:],
                op0=Alu.mult, op1=Alu.subtract)
            sd8 = persist.tile([128, 8], f32, name="sd8")
            nc.scalar.activation(out=sd8[:], in_=var8[:], func=Act.Sqrt,
                                 bias=eps_col[:])
            rstd8 = persist.tile([128, 8], f32, name="rstd8")
            nc.vector.reciprocal(rstd8[:], sd8[:])
            scale8 = persist.tile([128, 8], f32, name="scale8")
            nc.vector.tensor_tensor(out=scale8[:], in0=rstd8[:], in1=wc8[:],
                                    op=Alu.mult)
            q8 = persist.tile([128, 8], f32, name="q8")
            nc.vector.tensor_tensor(out=q8[:], in0=sm[:], in1=scale8[:],
                                    op=Alu.mult)
            shift8 = persist.tile([128, 8], f32, name="shift8")
            nc.vector.scalar_tensor_tensor(
                out=shift8[:], in0=q8[:], scalar=-1.0, in1=bc8[:],
                op0=Alu.mult, op1=Alu.add)

            bgate = smap("bgate", [128, CT * H], dtype=bf16)
            cgate = smap("cgate", [128, CT * W], dtype=bf16)
            for s in range(S):
                scc8 = cols.tile([128, 8], f32, name=f"scc8_{s}", tag="c8")
                nc.vector.tensor_tensor(out=scc8[:], in0=scale8[:],
                                        in1=itc8[s][:], op=Alu.mult)
                arg = gscr.tile([128, 8, 128], f32, name=f"arg{s}", tag="arg",
                                bufs=1)
                nc.vector.tensor_tensor(
                    out=arg[:],
                    in0=Y8[s][:].rearrange("p (m h) -> p m h", m=8),
                    in1=scc8[:].unsqueeze(2).broadcast_to([128, 8, 128]),
                    op=Alu.mult)
                nc.vector.tensor_tensor(
                    out=arg[:], in0=arg[:],
                    in1=shift8[:].unsqueeze(2).broadcast_to([128, 8, 128]),
                    op=Alu.add)
                gh = gscr.tile([128, 8, 128], bf16, name=f"gh{s}", tag="gh",
                               bufs=1)
                nc.scalar.activation(out=gh[:], in_=arg[:], func=Act.Sigmoid)
                nc.vector.tensor_tensor(
                    out=bgate[s][:].rearrange("p (t h) -> p t h", t=CT),
                    in0=gh[:, 0:2, :], in1=gh[:, 2:4, :], op=Alu.mult)
                nc.vector.tensor_tensor(
                    out=cgate[s][:].rearrange("p (t w) -> p t w", t=CT),
                    in0=gh[:, 4:6, :], in1=gh[:, 6:8, :], op=Alu.mult)

        # ---------------- phase E: apply (partition = channel) ----------------
        # out = (1 + A*B*C) * x, per chunk:
        #   mCA = afk * C[mid-bcast]          (DVE 2x)
        #   bexp = B[inner-bcast]             (Act copy)
        #   m = mCA * bexp                    (DVE 2x)
        #   m += 1                            (DVE 4x tensor_scalar)
        #   o = m * xres                      (DVE 2x)
        if "E" not in phases:
            return
        e_stack = contextlib.ExitStack()
        with e_stack:
            pme = e_stack.enter_context(tc.tile_pool(name="pme", bufs=2))
            paf = e_stack.enter_context(tc.tile_pool(name="paf", bufs=1))

            for s in range(S):
                # replicate A = g12*g13 [h,w] across the 128 c partitions
                # via stride-0 partition-broadcast DMA loads, one per h-chunk;
                # k-outer order so each A chunk serves both c tiles and the
                # replicas rotate through a single 3-buffer tag
                for k in range(NCH):
                    af = paf.tile([128, HC, W], bf16, name=f"af{s}{k}",
                                  tag="af", bufs=2)
                    nc.sync.dma_start(
                        af[:], adram[s].rearrange("(h w) -> h w", h=H)
                        [k * HC:(k + 1) * HC, :]
                        .unsqueeze(0).broadcast_to([128, HC, W]))
                    for t in range(CT):
                        csl = cgate[s][:, t * W:(t + 1) * W] \
                            .unsqueeze(1).broadcast_to([128, HC, W])
                        xsrc = xres[s][t][:, k * HC:(k + 1) * HC, :]
                        mca = pme.tile([128, HC, W], bf16,
                                       name=f"mca{s}{t}{k}", tag="mca")
                        nc.vector.tensor_tensor(out=mca[:], in0=af[:],
                                                in1=csl, op=Alu.mult)
                        bexp = pme.tile([128, HC, W], bf16,
                                        name=f"be{s}{t}{k}", tag="bexp")
                        bsl = bgate[s][:, t * H + k * HC:
                                       t * H + (k + 1) * HC] \
                            .unsqueeze(2).broadcast_to([128, HC, W])
                        if k % 2:
                            nc.gpsimd.tensor_copy(bexp[:], bsl)
                        else:
                            nc.scalar.copy(bexp[:], bsl)
                        m = pme.tile([128, HC, W], bf16, name=f"m{s}{t}{k}",
                                     tag="m")
                        nc.vector.tensor_tensor(out=m[:], in0=mca[:],
                                                in1=bexp[:], op=Alu.mult)
                        mp = pme.tile([128, HC, W], bf16, name=f"mp{s}{t}{k}",
                                      tag="mp")
                        if k % 2:
                            nc.vector.tensor_scalar_add(mp[:], m[:], 1.0)
                        else:
                            # m+1 on Act: Copy(1.0*m + 1.0)
                            nc.scalar.activation(out=mp[:], in_=m[:],
                                                 func=Act.Copy, bias=1.0)
                        o = pme.tile([128, HC, W], bf16, name=f"oe{s}{t}{k}",
                                     tag="oe")
                        nc.vector.tensor_tensor(out=o[:], in0=mp[:], in1=xsrc,
                                                op=Alu.mult)
                        nc.scalar.dma_start(
                            outy[s, t * 128:(t + 1) * 128,
                                 k * HC:(k + 1) * HC, :], o[:])


_NC_CACHE = {}
LAST_RESULT = None


def _get_nc(n_cores: int, sync_start: bool = False, phases: str = "ABCDE"):
    key = (n_cores, sync_start, phases)
    if key not in _NC_CACHE:
        _NC_CACHE[key] = build_bass(n_cores, sync_start, phases)
    return _NC_CACHE[key]


def kernel(**inputs) -> np.ndarray:
    from concourse.bass_utils import run_bass_kernel_spmd

    x = np.ascontiguousarray(inputs["x"], dtype=np.float32)
    bn1_w = np.ascontiguousarray(inputs["bn1_w"], dtype=np.float32)
    bn1_b = np.ascontiguousarray(inputs["bn1_b"], dtype=np.float32)
    bnc_w = np.ascontiguousarray(inputs["bnc_w"], dtype=np.float32)
    bnc_b = np.ascontiguousarray(inputs["bnc_b"], dtype=np.float32)
    B = x.shape[0]
    assert B == NCORES * S, (B, NCORES, S)

    nc = _get_nc(NCORES)
    in_maps = []
    for i in range(NCORES):
        in_maps.append({
            "xs": np.ascontiguousarray(x[i * S:(i + 1) * S]),
            "bn1_w": bn1_w, "bn1_b": bn1_b,
            "bnc_w": bnc_w, "bnc_b": bnc_b,
        })
    res = run_bass_kernel_spmd(nc, in_maps, core_ids=list(range(NCORES)))
    global LAST_RESULT
    LAST_RESULT = res
    out = np.concatenate(
        [np.asarray(res.results[i]["outy"]).astype(np.float32)
         for i in range(NCORES)], axis=0)
    return out


# revision 16
# speedup vs baseline: 22.7598x; 2.0233x over previous
"""Trainium2 Bass kernel for nn_Cross_Attention (triplet-pool cross-attention gating).

Math (per sample b):
  pools:  Shw[h,w]=max_c x,  Sch[c,h]=max_w x,  Scw[c,w]=max_h x
  3 branches of flat-softmax cross attention between pools -> y12,y13 [h,w],
  y21,y23 [c,h], y31,y32 [c,w]
  training-mode BatchNorm over the *global* batch (cross-core allreduce of
  sum/sumsq), sigmoid gates, and finally
  out = x * (g12*g13)[h,w] * (g21*g23)[c,h] * (g31*g32)[c,w] + x
      = x * (1 + A[h,w]*B[c,h]*Cg[c,w])

Sharding: batch-parallel, 2 samples per core on 8 cores; only the BN batch
stats cross cores (AllReduce of a [128,20] tile).

Engine split (v2): x is converted to a resident bf16 copy on the Act engine;
all three pools come from that copy -- Sch/Scw via DVE tensor_tensor max
trees (bf16 packed operands run the DVE at 2x), Shw via a gpsimd pair-max +
cross-partition tensor_reduce (axis=C) with a small SBUF->SBUF DMA to lay the
rows out partition=h.  The apply phase is three 2x TT ops plus a 4x
tensor-scalar (+1): m = (A*C)[mid-bcast] * B_exp, out = (m+1)*x, where B_exp
is materialised by the Act engine (innermost-stride-0 operands would drop the
DVE to 1x).
"""

import numpy as np

import concourse.bacc as bacc
import concourse.mybir as mybir
import concourse.tile as tile
from concourse import masks
import concourse.bass_isa as bass_isa

f32 = mybir.dt.float32
bf16 = mybir.dt.bfloat16
Alu = mybir.AluOpType
Act = mybir.ActivationFunctionType
X = mybir.AxisListType.X
CAX = mybir.AxisListType.C

NCORES = 8
S = 2          # samples per core
C, H, W = 256, 128, 128
CT = 2         # c tiles of 128
HC = 16        # h rows per chunk
NCH = H // HC  # 8
NP = 20        # bnp columns
EPS = 1e-5


def build_bass(n_cores: int, sync_start: bool = False, phases: str = "ABCDE",
               reps: int = 1):
    """sync_start/phases are for timing probes only: sync_start prepends a
    tiny AllReduce so all cores start main work in lockstep (makes full
    device time visible to the marginal-time harness); phases truncates."""
    nc = bacc.Bacc("TRN2", target_bir_lowering=False, debug=False,
                   num_devices=n_cores)
    nb_tot = n_cores * S
    n1 = float(nb_tot * H * W)   # bn1 count
    ncn = float(nb_tot * H)      # bnc count (per channel)

    xs = nc.dram_tensor("xs", [S, C, H, W], f32, kind="ExternalInput").ap()
    bn1w = nc.dram_tensor("bn1_w", [1], f32, kind="ExternalInput").ap()
    bn1b = nc.dram_tensor("bn1_b", [1], f32, kind="ExternalInput").ap()
    bncw = nc.dram_tensor("bnc_w", [C], f32, kind="ExternalInput").ap()
    bncb = nc.dram_tensor("bnc_b", [C], f32, kind="ExternalInput").ap()
    outy = nc.dram_tensor("outy", [S, C, H, W], bf16,
                          kind="ExternalOutput").ap()

    ccin = nc.dram_tensor("ccin", [128, NP], f32).ap()
    ccout = nc.dram_tensor(
        "ccout", [128, NP], f32,
        addr_space="Shared" if n_cores > 1 else "Local").ap()
    adram = nc.dram_tensor("adram", [S, H * W], bf16).ap()
    srow = nc.dram_tensor("srow", [S, H * W], bf16).ap()
    sync_bufs = None
    if sync_start:
        sin = nc.dram_tensor("sin", [1, 1], f32).ap()
        sout = nc.dram_tensor(
            "sout", [1, 1], f32,
            addr_space="Shared" if n_cores > 1 else "Local").ap()
        sync_bufs = (sin, sout)

    with tile.TileContext(nc) as tc:
        for rep in range(reps):
            _emit(nc, tc, n_cores, n1, ncn,
                  xs, bn1w, bn1b, bncw, bncb, outy, ccin, ccout, adram, srow,
                  sync_bufs, phases, rep)
    nc.compile()
    return nc


def _emit(nc, tc, n_cores, n1, ncn,
          xs, bn1w, bn1b, bncw, bncb, outy, ccin, ccout, adram, srow,
          sync_bufs=None, phases="ABCDE", rep=0):
    import contextlib
    stack = contextlib.ExitStack()
    with stack:
        persist = stack.enter_context(tc.tile_pool(name=f"persist{rep}", bufs=1))
        maps = stack.enter_context(tc.tile_pool(name=f"maps{rep}", bufs=2))
        cols = stack.enter_context(tc.tile_pool(name=f"cols{rep}", bufs=4))
        keep = stack.enter_context(tc.tile_pool(name=f"keep{rep}", bufs=1))
        gscr = stack.enter_context(tc.tile_pool(name=f"gscr{rep}", bufs=4))

        # --- timing-only start barrier: a tiny AllReduce whose result is
        # loaded on the sync DMA queue, so every later HWDGE load (FIFO per
        # engine) waits until all cores have started this iteration ---
        if sync_bufs is not None:
            sin, sout = sync_bufs
            st0 = cols.tile([1, 1], f32, name="st0", tag="c0")
            nc.vector.memset(st0[:], 1.0)
            nc.sync.dma_start(sin, st0[:])
            nc.gpsimd.collective_compute(
                "AllReduce", Alu.add,
                replica_groups=[list(range(n_cores))],
                ins=[sin], outs=[sout])
            st1 = cols.tile([1, 1], f32, name="st1", tag="c0")
            nc.sync.dma_start(st1[:], sout)
        else:
            st1 = None

        # --- setup ---
        identity = persist.tile([128, 128], f32)
        masks.make_identity(nc, identity[:])
        identb = persist.tile([128, 128], bf16)
        nc.vector.tensor_copy(identb[:], identity[:])
        ones_r = persist.tile([1, 128], f32)
        nc.vector.memset(ones_r[:], 1.0)
        ones_c = persist.tile([128, 1], f32)
        nc.vector.memset(ones_c[:], 1.0)
        eps_col = persist.tile([128, 1], f32)
        nc.vector.memset(eps_col[:], EPS)
        wc2 = persist.tile([128, 2], f32)
        nc.sync.dma_start(wc2[:], bncw.rearrange("(t c) -> c t", c=128))
        bc2 = persist.tile([128, 2], f32)
        nc.sync.dma_start(bc2[:], bncb.rearrange("(t c) -> c t", c=128))
        bn1w_sb = persist.tile([1, 1], f32)
        nc.sync.dma_start(bn1w_sb[:], bn1w.unsqueeze(1))
        bn1b_sb = persist.tile([1, 1], f32)
        nc.sync.dma_start(bn1b_sb[:], bn1b.unsqueeze(1))
        wc8 = persist.tile([128, 8], f32)
        bc8 = persist.tile([128, 8], f32)
        for m in range(4):
            nc.vector.tensor_copy(wc8[:, m * 2:m * 2 + 2], wc2[:])
            nc.vector.tensor_copy(bc8[:, m * 2:m * 2 + 2], bc2[:])
        bnpS = []
        for s in range(S):
            t_ = persist.tile([128, NP], f32, name=f"bnpS{s}")
            nc.vector.memset(t_[:], 0.0)
            bnpS.append(t_)
        bnp = persist.tile([128, NP], f32)

        # per-sample persistent maps (bufs=2 -> one slot per sample)
        def smap(name, shape, bufs=None, dtype=f32):
            return [maps.tile(shape, dtype, name=f"{name}{s}", tag=name,
                              bufs=bufs)
                    for s in range(S)]

        xch = smap("xch", [128, CT * H], dtype=bf16)    # [c_loc, (t,h)]
        xcw = smap("xcw", [128, CT * W], dtype=bf16)    # [c_loc, (t,w)]
        xhwT = smap("xhwT", [128, H], dtype=bf16)       # [w, h]
        shw = smap("shw", [128, W], dtype=bf16)         # [h, w]
        y12 = smap("y12", [128, W])         # [h, w] f32
        y13 = smap("y13", [128, W])         # [h, w] f32
        # the four bnc y-maps live in one [128, (m,t)*128] tile per sample,
        # cols (m, t): m0=y21(br12) m1=y23(br23) m2=y31(br13) m3=y32(br23),
        # so the whole B/C gate batch is one sigmoid activation later
        Y8 = smap("Y8", [128, 8 * 128])

        def ymt(s, m, t):
            return Y8[s][:, (2 * m + t) * 128:(2 * m + t + 1) * 128]

        def ym(s, m):
            return Y8[s][:, 2 * m * 128:(2 * m + 2) * 128]
        agate = smap("agate", [128, W], dtype=bf16)     # [h, w]
        itc = {}   # invT cols [128,1] per (s, branch)
        it1 = {}   # invT [1,1] per (s, branch)

        # resident bf16 x: [c_loc, h, w] per (s, t) -- conversion on Act,
        # source for all pools and the apply phase
        xres = [[persist.tile([128, H, W], bf16,
                              name=f"xres{s}{t}", tag=f"xres{s}{t}")
                 for t in range(CT)] for s in range(S)]

        ps_stack = contextlib.ExitStack()
        with ps_stack:
            ptree = ps_stack.enter_context(tc.tile_pool(name=f"ptree{rep}", bufs=1))
            ppm = ps_stack.enter_context(tc.tile_pool(name=f"ppm{rep}", bufs=2))
            pbm = ps_stack.enter_context(tc.tile_pool(name=f"pbm{rep}", bufs=1))

            def bmap(name, shape, dtype=f32):
                return [pbm.tile(shape, dtype, name=f"{name}{s}", tag=name)
                        for s in range(S)]

            e12 = bmap("e12", [128, C], bf16)         # [w, c]
            e12t = bmap("e12t", [128, CT * W], bf16)  # [c_loc, (t,w)]
            e13 = bmap("e13", [128, C], bf16)         # [h, c]
            e13t = bmap("e13t", [128, CT * H], bf16)  # [c_loc, (t,h)]
            e23 = bmap("e23", [128, W], bf16)         # [h, w]
            e23t = bmap("e23t", [128, H], bf16)       # [w, h]
            y12T = bmap("y12T", [128, H])             # [w, h] f32
            ps_mm = ps_stack.enter_context(
                tc.tile_pool(name=f"ps_mm{rep}", bufs=2, space="PSUM"))
            ps_ty = ps_stack.enter_context(
                tc.tile_pool(name=f"ps_ty{rep}", bufs=2, space="PSUM"))

            # ---------------- pass 1: load, convert, pools ----------------
            def emit_pass1(s):
                HH = H // 2
                for half in range(2):
                    for t in range(CT):
                        # casting DMA (gpsimd-initiated SWDGE): f32 DRAM ->
                        # bf16 resident copy, half a c-tile per trigger to
                        # amortise the SWDGE fixed cost on the Pool queue
                        dst = xres[s][t][:, half * HH:(half + 1) * HH, :]
                        if st1 is not None:
                            # timing probe: force every load to wait for the
                            # start barrier via a WAW dep on the tile
                            nc.vector.tensor_copy(
                                xres[s][t][0:1, half * HH, 0:1], st1[:])
                        nc.gpsimd.dma_start(
                            dst, xs[s, t * 128:(t + 1) * 128,
                                    half * HH:(half + 1) * HH, :])
                    for k in range(half * (NCH // 2), (half + 1) * (NCH // 2)):
                        # Shw: pair-max over the two c tiles (alternating
                        # DVE/Pool), then max over the 128 c_loc partitions
                        # (Pool, axis=C), then a DRAM bounce to partition=h
                        pm = ppm.tile([128, HC, W], bf16, name=f"pm{s}{k}",
                                      tag="pm")
                        # neuronxcc rejects TensorTensor on Pool; DVE 2x
                        nc.vector.tensor_tensor(
                            out=pm[:],
                            in0=xres[s][0][:, k * HC:(k + 1) * HC, :],
                            in1=xres[s][1][:, k * HC:(k + 1) * HC, :],
                            op=Alu.max)
                        row = ppm.tile([128, HC * W], bf16,
                                       name=f"row{s}{k}", tag="row")
                        nc.gpsimd.partition_all_reduce(
                            row[:], pm[:].rearrange("p h w -> p (h w)"),
                            channels=128, reduce_op=bass_isa.ReduceOp.max)
                        nc.sync.dma_start(
                            srow[s, k * HC * W:(k + 1) * HC * W].unsqueeze(0),
                            row[0:1, :])
                # Shw rows land partition=h via a DRAM bounce
                nc.sync.dma_start(
                    shw[s][:], srow[s].rearrange("(h w) -> h w", h=H))

                # Sch / Scw via DVE bf16 max trees (2x packed mode).
                # Quarter-fold first so every scratch tile is <= 8KB and all
                # levels cycle through one shared 3-buffer tag.
                def tscr(shape, nm):
                    return ptree.tile(shape, bf16, name=nm, tag="tr", bufs=3)

                for t in range(CT):
                    xr = xres[s][t]
                    # Scw: fold h 128 -> 1
                    q1 = tscr([128, 32, W], f"cwq1{s}{t}")
                    nc.vector.tensor_tensor(out=q1[:], in0=xr[:, 0:32, :],
                                            in1=xr[:, 64:96, :], op=Alu.max)
                    q2 = tscr([128, 32, W], f"cwq2{s}{t}")
                    nc.vector.tensor_tensor(out=q2[:], in0=xr[:, 32:64, :],
                                            in1=xr[:, 96:128, :], op=Alu.max)
                    cur = tscr([128, 32, W], f"cwm32{s}{t}")
                    nc.vector.tensor_tensor(out=cur[:], in0=q1[:], in1=q2[:],
                                            op=Alu.max)
                    hs = 16
                    while hs >= 1:
                        if hs == 1:
                            dst_ap = xcw[s][:, t * W:(t + 1) * W].unsqueeze(1)
                        else:
                            nt = tscr([128, hs, W], f"cw{s}{t}{hs}")
                            dst_ap = nt[:]
                        nc.vector.tensor_tensor(
                            out=dst_ap, in0=cur[:, 0:hs, :],
                            in1=cur[:, hs:2 * hs, :], op=Alu.max)
                        if hs > 1:
                            cur = nt
                        hs //= 2
                    # Sch: fold w 128 -> 1
                    q1 = tscr([128, H, 32], f"chq1{s}{t}")
                    nc.vector.tensor_tensor(out=q1[:], in0=xr[:, :, 0:32],
                                            in1=xr[:, :, 64:96], op=Alu.max)
                    q2 = tscr([128, H, 32], f"chq2{s}{t}")
                    nc.vector.tensor_tensor(out=q2[:], in0=xr[:, :, 32:64],
                                            in1=xr[:, :, 96:128], op=Alu.max)
                    cur = tscr([128, H, 32], f"chm32{s}{t}")
                    nc.vector.tensor_tensor(out=cur[:], in0=q1[:], in1=q2[:],
                                            op=Alu.max)
                    ws = 16
                    while ws >= 1:
                        if ws == 1:
                            dst_ap = xch[s][:, t * H:(t + 1) * H].unsqueeze(2)
                        else:
                            nt = tscr([128, H, ws], f"ch{s}{t}{ws}")
                            dst_ap = nt[:]
                        nc.vector.tensor_tensor(
                            out=dst_ap, in0=cur[:, :, 0:ws],
                            in1=cur[:, :, ws:2 * ws], op=Alu.max)
                        if ws > 1:
                            cur = nt
                        ws //= 2

                # xhwT = transpose(shw)
                tp = ps_ty.tile([128, 128], bf16, name=f"shwT{s}", tag="tyb")
                nc.tensor.transpose(tp[:], shw[s][:], identb[:])
                nc.vector.tensor_copy(xhwT[s][:], tp[:])

            # ---------------- phase B: attention ----------------
            def psum_copy_to(dst, src_ps):
                nc.scalar.copy(dst, src_ps)

            def transpose_to(dst, src_sb, nblk, name):
                # src [128, nblk*128] bf16 -> dst [128, nblk*128] blockwise T
                for t in range(nblk):
                    tp = ps_mm.tile([128, 128], bf16, name=f"tp{name}{t}",
                                    tag="mmb")
                    nc.tensor.transpose(
                        tp[:], src_sb[:, t * 128:(t + 1) * 128], identb[:])
                    psum_copy_to(dst[:, t * 128:(t + 1) * 128], tp[:])

            def softmax(s, br, sim_ps, ncol, e_dst):
                rowmax = cols.tile([128, 1], f32, name=f"rm{s}{br}", tag="c1")
                nc.vector.tensor_reduce(out=rowmax[:], in_=sim_ps[:], axis=X,
                                        op=Alu.max)
                rmt = ps_ty.tile([1, 128], f32, name=f"rmt{s}{br}", tag="ty")
                nc.tensor.transpose(rmt[:], rowmax[:], identity[:])
                ngmax = cols.tile([1, 1], f32, name=f"ngm{s}{br}", tag="c0")
                nc.vector.tensor_reduce(out=ngmax[:], in_=rmt[:], axis=X,
                                        op=Alu.max, negate=True)
                nm_ps = ps_ty.tile([128, 1], f32, name=f"nmp{s}{br}", tag="ty")
                nc.tensor.matmul(nm_ps[:], ones_r[:], ngmax[:])
                nmcol = cols.tile([128, 1], f32, name=f"nmc{s}{br}", tag="c1")
                psum_copy_to(nmcol[:], nm_ps[:])
                rowsum = cols.tile([128, 1], f32, name=f"rs{s}{br}", tag="c1")
                nc.scalar.activation(out=e_dst[:], in_=sim_ps[:], func=Act.Exp,
                                     bias=nmcol[:], scale=1.0,
                                     accum_out=rowsum[:])
                tot_ps = ps_ty.tile([1, 1], f32, name=f"tot{s}{br}", tag="ty")
                nc.tensor.matmul(tot_ps[:], rowsum[:], ones_c[:])
                invt = keep.tile([1, 1], f32, name=f"it{s}{br}",
                                 tag=f"it{s}{br}")
                nc.vector.reciprocal(invt[:], tot_ps[:])
                ic_ps = ps_ty.tile([128, 1], f32, name=f"icp{s}{br}", tag="ty")
                nc.tensor.matmul(ic_ps[:], ones_r[:], invt[:])
                iccol = keep.tile([128, 1], f32, name=f"icc{s}{br}",
                                  tag=f"icc{s}{br}")
                psum_copy_to(iccol[:], ic_ps[:])
                it1[(s, br)] = invt
                itc[(s, br)] = iccol

            scht = bmap("scht", [128, CT * H], bf16)  # [h, (t,c_loc)]
            scwt = bmap("scwt", [128, CT * W], bf16)  # [w, (t,c_loc)]

            def emit_phaseB(s):
                transpose_to(scht[s], xch[s], CT, f"sch{s}")
                transpose_to(scwt[s], xcw[s], CT, f"scw{s}")

                # --- branch 12: sim12[w,c] = sum_h Shw[h,w] Sch[c,h]
                sim12 = ps_mm.tile([128, C], f32, name=f"s12_{s}", tag="mm")
                nc.tensor.matmul(sim12[:], shw[s][:], scht[s][:])
                softmax(s, 12, sim12, C, e12[s])
                transpose_to(e12t[s], e12[s], CT, f"e12{s}")
                # y12T[w,h] = sum_c e12t[c,w]^T ... accumulate 2 c tiles
                y12p = ps_mm.tile([128, H], f32, name=f"y12p{s}", tag="mm")
                for t in range(CT):
                    nc.tensor.matmul(
                        y12p[:], e12t[s][:, t * W:(t + 1) * W],
                        xch[s][:, t * H:(t + 1) * H],
                        start=(t == 0), stop=(t == CT - 1))
                psum_copy_to(y12T[s][:], y12p[:])
                # y21[c,h] per c tile
                for t in range(CT):
                    y21p = ps_mm.tile([128, H], f32, name=f"y21p{s}{t}",
                                      tag="mm")
                    nc.tensor.matmul(y21p[:], e12[s][:, t * 128:(t + 1) * 128],
                                     xhwT[s][:])
                    psum_copy_to(ymt(s, 0, t), y21p[:])

                # --- branch 13: sim13[h,c] = sum_w Shw[h,w] Scw[c,w]
                sim13 = ps_mm.tile([128, C], f32, name=f"s13_{s}", tag="mm")
                nc.tensor.matmul(sim13[:], xhwT[s][:], scwt[s][:])
                softmax(s, 13, sim13, C, e13[s])
                transpose_to(e13t[s], e13[s], CT, f"e13{s}")
                y13p = ps_mm.tile([128, W], f32, name=f"y13p{s}", tag="mm")
                for t in range(CT):
                    nc.tensor.matmul(
                        y13p[:], e13t[s][:, t * H:(t + 1) * H],
                        xcw[s][:, t * W:(t + 1) * W],
                        start=(t == 0), stop=(t == CT - 1))
                psum_copy_to(y13[s][:], y13p[:])
                for t in range(CT):
                    y31p = ps_mm.tile([128, W], f32, name=f"y31p{s}{t}",
                                      tag="mm")
                    nc.tensor.matmul(y31p[:], e13[s][:, t * 128:(t + 1) * 128],
                                     shw[s][:])
                    psum_copy_to(ymt(s, 2, t), y31p[:])

                # --- branch 23: sim23[h,w] = sum_c Sch[c,h] Scw[c,w]
                sim23 = ps_mm.tile([128, W], f32, name=f"s23_{s}", tag="mm")
                for t in range(CT):
                    nc.tensor.matmul(
                        sim23[:], xch[s][:, t * H:(t + 1) * H],
                        xcw[s][:, t * W:(t + 1) * W],
                        start=(t == 0), stop=(t == CT - 1))
                softmax(s, 23, sim23, W, e23[s])
                transpose_to(e23t[s], e23[s], 1, f"e23{s}")
                for t in range(CT):
                    y23p = ps_mm.tile([128, H], f32, name=f"y23p{s}{t}",
                                      tag="mm")
                    nc.tensor.matmul(y23p[:], scwt[s][:, t * W:(t + 1) * W],
                                     e23t[s][:])
                    psum_copy_to(ymt(s, 1, t), y23p[:])
                    y32p = ps_mm.tile([128, W], f32, name=f"y32p{s}{t}",
                                      tag="mm")
                    nc.tensor.matmul(y32p[:], scht[s][:, t * H:(t + 1) * H],
                                     e23[s][:])
                    psum_copy_to(ymt(s, 3, t), y32p[:])

                # y12 = transpose(y12T)  (f32)
                y12pp = ps_mm.tile([128, 128], f32, name=f"y12pp{s}", tag="mm")
                nc.tensor.transpose(y12pp[:], y12T[s][:], identity[:])
                psum_copy_to(y12[s][:], y12pp[:])

            # ---------------- phase C: BN partials ----------------
            ysq = gscr.tile([128, 128], f32, name="ysq", tag="ysq", bufs=2)

            def emit_phaseC(s):
                bnp = bnpS[s]
                it2 = {}
                for br in (12, 13, 23):
                    t2 = keep.tile([128, 1], f32, name=f"it2_{s}{br}",
                                   tag=f"it2_{s}{br}")
                    nc.vector.tensor_tensor(out=t2[:], in0=itc[(s, br)][:],
                                            in1=itc[(s, br)][:], op=Alu.mult)
                    it2[br] = t2
                bnc_maps = [(0, 12), (1, 23), (2, 13), (3, 23)]
                for m, br in bnc_maps:
                    r2 = cols.tile([128, 2], f32, name=f"r{s}{m}", tag="c2")
                    nc.vector.tensor_reduce(
                        out=r2[:], in_=ym(s, m).rearrange("p (t h) -> p t h",
                                                          t=CT),
                        axis=X, op=Alu.add)
                    nc.vector.scalar_tensor_tensor(
                        out=bnp[:, m * 2:m * 2 + 2], in0=r2[:],
                        scalar=itc[(s, br)][:], in1=bnp[:, m * 2:m * 2 + 2],
                        op0=Alu.mult, op1=Alu.add)
                    for t in range(CT):
                        col = m * 2 + t
                        blk = ymt(s, m, t)
                        sq = cols.tile([128, 1], f32, name=f"sq{s}{m}{t}",
                                       tag="c1")
                        nc.scalar.activation(out=ysq[:], in_=blk,
                                             func=Act.Square, accum_out=sq[:])
                        nc.vector.scalar_tensor_tensor(
                            out=bnp[:, 8 + col:9 + col], in0=sq[:],
                            scalar=it2[br][:], in1=bnp[:, 8 + col:9 + col],
                            op0=Alu.mult, op1=Alu.add)
                # bn1 partials (partition 0, cols 16..19)
                for j, (ymap, br) in enumerate(((y12T[s], 12), (y13[s], 13))):
                    i1 = it1[(s, br)]
                    i2 = cols.tile([1, 1], f32, name=f"i2_{s}{j}", tag="c0")
                    nc.vector.tensor_tensor(out=i2[:], in0=i1[:], in1=i1[:],
                                            op=Alu.mult)
                    rs = cols.tile([128, 1], f32, name=f"rs1_{s}{j}", tag="c1")
                    nc.vector.tensor_reduce(out=rs[:], in_=ymap[:], axis=X,
                                            op=Alu.add)
                    tp = ps_ty.tile([1, 1], f32, name=f"t1_{s}{j}", tag="ty")
                    nc.tensor.matmul(tp[:], rs[:], ones_c[:])
                    nc.vector.scalar_tensor_tensor(
                        out=bnp[0:1, 16 + 2 * j:17 + 2 * j], in0=tp[:],
                        scalar=i1[:], in1=bnp[0:1, 16 + 2 * j:17 + 2 * j],
                        op0=Alu.mult, op1=Alu.add)
                    sqc = cols.tile([128, 1], f32, name=f"sqc{s}{j}", tag="c1")
                    nc.scalar.activation(out=ysq[:], in_=ymap[:],
                                         func=Act.Square, accum_out=sqc[:])
                    tp2 = ps_ty.tile([1, 1], f32, name=f"t2_{s}{j}", tag="ty")
                    nc.tensor.matmul(tp2[:], sqc[:], ones_c[:])
                    nc.vector.scalar_tensor_tensor(
                        out=bnp[0:1, 17 + 2 * j:18 + 2 * j], in0=tp2[:],
                        scalar=i2[:], in1=bnp[0:1, 17 + 2 * j:18 + 2 * j],
                        op0=Alu.mult, op1=Alu.add)

            # interleaved emission: sample s's attention + BN partials
            # overlap the next sample's bulk pass-1 work in the in-order
            # engine queues
            for s in range(S):
                if "A" not in phases:
                    break
                emit_pass1(s)
                if "B" in phases:
                    emit_phaseB(s)
                    if "C" in phases:
                        emit_phaseC(s)
            if ("A" not in phases or "B" not in phases
                    or "C" not in phases):
                return

            # pre-collective: per-sample itc columns in the (m,t) layout
            itc8 = []
            for s in range(S):
                t8 = keep.tile([128, 8], f32, name=f"itc8_{s}",
                               tag=f"itc8_{s}")
                for m, br in enumerate((12, 23, 13, 23)):
                    nc.vector.tensor_copy(
                        t8[:, 2 * m:2 * m + 2],
                        itc[(s, br)][:].broadcast_to([128, 2]))
                itc8.append(t8)

            # ---------------- allreduce ----------------
            nc.vector.tensor_tensor(out=bnp[:], in0=bnpS[0][:],
                                    in1=bnpS[1][:], op=Alu.add)
            nc.sync.dma_start(ccin, bnp[:])
            if n_cores > 1:
                nc.gpsimd.collective_compute(
                    "AllReduce", Alu.add,
                    replica_groups=[list(range(n_cores))],
                    ins=[ccin], outs=[ccout])
            else:
                nc.sync.dma_start(ccout, ccin)
            bnpg = persist.tile([128, NP], f32)
            nc.sync.dma_start(bnpg[:], ccout)

            # ---------------- phase D: BN finalize + gates ----------------
            # bn1 scalar chain first so the A gates (and the afull broadcast
            # DMA) unblock phase E as early as possible
            sc1 = []
            sh1 = []
            for j in range(2):
                mu1 = cols.tile([1, 1], f32, name=f"mu1_{j}", tag="c0")
                nc.vector.tensor_scalar_mul(
                    mu1[:], bnpg[0:1, 16 + 2 * j:17 + 2 * j], 1.0 / n1)
                m21 = cols.tile([1, 1], f32, name=f"m21_{j}", tag="c0")
                nc.vector.tensor_tensor(out=m21[:], in0=mu1[:], in1=mu1[:],
                                        op=Alu.mult)
                v1 = cols.tile([1, 1], f32, name=f"v1_{j}", tag="c0")
                nc.vector.scalar_tensor_tensor(
                    out=v1[:], in0=bnpg[0:1, 17 + 2 * j:18 + 2 * j],
                    scalar=1.0 / n1, in1=m21[:], op0=Alu.mult,
                    op1=Alu.subtract)
                sd1 = cols.tile([1, 1], f32, name=f"sd1_{j}", tag="c0")
                nc.scalar.activation(out=sd1[:], in_=v1[:], func=Act.Sqrt,
                                     bias=eps_col[0:1, :])
                rst1 = cols.tile([1, 1], f32, name=f"rst1_{j}", tag="c0")
                nc.vector.reciprocal(rst1[:], sd1[:])
                sc = keep.tile([1, 1], f32, name=f"sc1_{j}", tag=f"sc1_{j}")
                nc.vector.tensor_tensor(out=sc[:], in0=rst1[:],
                                        in1=bn1w_sb[:], op=Alu.mult)
                sc1.append(sc)
                q1 = cols.tile([1, 1], f32, name=f"q1_{j}", tag="c0")
                nc.vector.tensor_tensor(out=q1[:], in0=mu1[:], in1=sc[:],
                                        op=Alu.mult)
                sh = keep.tile([1, 1], f32, name=f"sh1_{j}", tag=f"sh1_{j}")
                nc.vector.scalar_tensor_tensor(
                    out=sh[:], in0=q1[:], scalar=-1.0, in1=bn1b_sb[:],
                    op0=Alu.mult, op1=Alu.add)
                sh1.append(sh)

            for s in range(S):
                rhs4 = cols.tile([1, 4], f32, name=f"rhs4_{s}", tag="c4")
                for j, br in ((0, 12), (1, 13)):
                    nc.vector.tensor_tensor(out=rhs4[:, j:j + 1],
                                            in0=sc1[j][:],
                                            in1=it1[(s, br)][:], op=Alu.mult)
                    nc.vector.tensor_copy(rhs4[:, 2 + j:3 + j], sh1[j][:])
                cm_ps = ps_ty.tile([128, 4], f32, name=f"cmp{s}", tag="ty")
                nc.tensor.matmul(cm_ps[:], ones_r[:], rhs4[:])
                colmat = cols.tile([128, 4], f32, name=f"cm{s}", tag="c4b")
                psum_copy_to(colmat[:], cm_ps[:])
                g1 = gscr.tile([128, W], bf16, name=f"g12_{s}", tag="ga")
                g2 = gscr.tile([128, W], bf16, name=f"g13_{s}", tag="ga")
                for j, (ymap, g) in enumerate(((y12[s], g1), (y13[s], g2))):
                    nc.scalar.activation(out=g[:], in_=ymap[:],
                                         func=Act.Sigmoid,
                                         bias=colmat[:, 2 + j:3 + j],
                                         scale=colmat[:, j:j + 1])
                nc.vector.tensor_tensor(out=agate[s][:], in0=g1[:], in1=g2[:],
                                        op=Alu.mult)
                nc.scalar.dma_start(
                    adram[s].rearrange("(h w) -> h w", h=H), agate[s][:])

            # bnc finalize (batched over all 4 maps x 2 c-tiles)
            sm = persist.tile([128, 8], f32, name="mu8")
            nc.vector.tensor_scalar_mul(sm[:], bnpg[:, 0:8], 1.0 / ncn)
            m2 = persist.tile([128, 8], f32, name="m28")
            nc.scalar.activation(out=m2[:], in_=bnpg[:, 0:8], func=Act.Square,
                                 scale=1.0 / ncn)
            var8 = persist.tile([128, 8], f32, name="var8")
            nc.vector.scalar_tensor_tensor(
                out=var8[:], in0=bnpg[:, 8:16], scalar=1.0 / ncn, in1=m2[:],
                op0=Alu.mult, op1=Alu.subtract)
            sd8 = persist.tile([128, 8], f32, name="sd8")
            nc.scalar.activation(out=sd8[:], in_=var8[:], func=Act.Sqrt,
                                 bias=eps_col[:])
            rstd8 = persist.tile([128, 8], f32, name="rstd8")
            nc.vector.reciprocal(rstd8[:], sd8[:])
            scale8 = persist.tile([128, 8], f32, name="scale8")
            nc.vector.tensor_tensor(out=scale8[:], in0=rstd8[:], in1=wc8[:],
                                    op=Alu.mult)
            q8 = persist.tile([128, 8], f32, name="q8")
            nc.vector.tensor_tensor(out=q8[:], in0=sm[:], in1=scale8[:],
                                    op=Alu.mult)
            shift8 = persist.tile([128, 8], f32, name="shift8")
            nc.vector.scalar_tensor_tensor(
                out=shift8[:], in0=q8[:], scalar=-1.0, in1=bc8[:],
                op0=Alu.mult, op1=Alu.add)

            bgate = smap("bgate", [128, CT * H], dtype=bf16)
            cgate = smap("cgate", [128, CT * W], dtype=bf16)
            for s in range(S):
                scc8 = cols.tile([128, 8], f32, name=f"scc8_{s}", tag="c8")
                nc.vector.tensor_tensor(out=scc8[:], in0=scale8[:],
                                        in1=itc8[s][:], op=Alu.mult)
                arg = gscr.tile([128, 8, 128], f32, name=f"arg{s}", tag="arg",
                                bufs=1)
                nc.vector.tensor_tensor(
                    out=arg[:],
                    in0=Y8[s][:].rearrange("p (m h) -> p m h", m=8),
                    in1=scc8[:].unsqueeze(2).broadcast_to([128, 8, 128]),
                    op=Alu.mult)
                nc.vector.tensor_tensor(
                    out=arg[:], in0=arg[:],
                    in1=shift8[:].unsqueeze(2).broadcast_to([128, 8, 128]),
                    op=Alu.add)
                gh = gscr.tile([128, 8, 128], bf16, name=f"gh{s}", tag="gh",
                               bufs=1)
                nc.scalar.activation(out=gh[:], in_=arg[:], func=Act.Sigmoid)
                nc.vector.tensor_tensor(
                    out=bgate[s][:].rearrange("p (t h) -> p t h", t=CT),
                    in0=gh[:, 0:2, :], in1=gh[:, 2:4, :], op=Alu.mult)
                nc.vector.tensor_tensor(
                    out=cgate[s][:].rearrange("p (t w) -> p t w", t=CT),
                    in0=gh[:, 4:6, :], in1=gh[:, 6:8, :], op=Alu.mult)

        # ---------------- phase E: apply (partition = channel) ----------------
        # out = (1 + A*B*C) * x, per chunk:
        #   mCA = afk * C[mid-bcast]          (DVE 2x)
        #   bexp = B[inner-bcast]             (Act copy)
        #   m = mCA * bexp                    (DVE 2x)
        #   m += 1                            (DVE 4x tensor_scalar)
        #   o = m * xres                      (DVE 2x)
        if "E" not in phases:
            return
        e_stack = contextlib.ExitStack()
        with e_stack:
            pme = e_stack.enter_context(tc.tile_pool(name=f"pme{rep}", bufs=2))
            paf = e_stack.enter_context(tc.tile_pool(name=f"paf{rep}", bufs=1))

            for s in range(S):
                # replicate A = g12*g13 [h,w] across the 128 c partitions
                # via stride-0 partition-broadcast DMA loads, one per h-chunk;
                # k-outer order so each A chunk serves both c tiles and the
                # replicas rotate through a single 3-buffer tag
                for k in range(NCH):
                    af = paf.tile([128, HC, W], bf16, name=f"af{s}{k}",
                                  tag="af", bufs=2)
                    if "f" in phases:
                        nc.vector.memset(af[:], 1.0)
                    else:
                        nc.sync.dma_start(
                            af[:], adram[s].rearrange("(h w) -> h w", h=H)
                            [k * HC:(k + 1) * HC, :]
                            .unsqueeze(0).broadcast_to([128, HC, W]))
                    for t in range(CT):
                        csl = cgate[s][:, t * W:(t + 1) * W] \
                            .unsqueeze(1).broadcast_to([128, HC, W])
                        xsrc = xres[s][t][:, k * HC:(k + 1) * HC, :]
                        mca = pme.tile([128, HC, W], bf16,
                                       name=f"mca{s}{t}{k}", tag="mca")
                        nc.vector.tensor_tensor(out=mca[:], in0=af[:],
                                                in1=csl, op=Alu.mult)
                        bexp = pme.tile([128, HC, W], bf16,
                                        name=f"be{s}{t}{k}", tag="bexp")
                        bsl = bgate[s][:, t * H + k * HC:
                                       t * H + (k + 1) * HC] \
                            .unsqueeze(2).broadcast_to([128, HC, W])
                        if k % 2 and "g" not in phases:
                            nc.gpsimd.tensor_copy(bexp[:], bsl)
                        else:
                            nc.scalar.copy(bexp[:], bsl)
                        m = pme.tile([128, HC, W], bf16, name=f"m{s}{t}{k}",
                                     tag="m")
                        nc.vector.tensor_tensor(out=m[:], in0=mca[:],
                                                in1=bexp[:], op=Alu.mult)
                        mp = pme.tile([128, HC, W], bf16, name=f"mp{s}{t}{k}",
                                      tag="mp")
                        if k % 2:
                            nc.vector.tensor_scalar_add(mp[:], m[:], 1.0)
                        else:
                            # m+1 on Act: Copy(1.0*m + 1.0)
                            nc.scalar.activation(out=mp[:], in_=m[:],
                                                 func=Act.Copy, bias=1.0)
                        o = pme.tile([128, HC, W], bf16, name=f"oe{s}{t}{k}",
                                     tag="oe")
                        nc.vector.tensor_tensor(out=o[:], in0=mp[:], in1=xsrc,
                                                op=Alu.mult)
                        nc.scalar.dma_start(
                            outy[s, t * 128:(t + 1) * 128,
                                 k * HC:(k + 1) * HC, :], o[:])


_NC_CACHE = {}
LAST_RESULT = None


def _get_nc(n_cores: int, sync_start: bool = False, phases: str = "ABCDE",
            reps: int = 1):
    key = (n_cores, sync_start, phases, reps)
    if key not in _NC_CACHE:
        _NC_CACHE[key] = build_bass(n_cores, sync_start, phases, reps)
    return _NC_CACHE[key]


def kernel(**inputs) -> np.ndarray:
    from concourse.bass_utils import run_bass_kernel_spmd

    x = np.ascontiguousarray(inputs["x"], dtype=np.float32)
    bn1_w = np.ascontiguousarray(inputs["bn1_w"], dtype=np.float32)
    bn1_b = np.ascontiguousarray(inputs["bn1_b"], dtype=np.float32)
    bnc_w = np.ascontiguousarray(inputs["bnc_w"], dtype=np.float32)
    bnc_b = np.ascontiguousarray(inputs["bnc_b"], dtype=np.float32)
    B = x.shape[0]
    assert B == NCORES * S, (B, NCORES, S)

    nc = _get_nc(NCORES)
    in_maps = []
    for i in range(NCORES):
        in_maps.append({
            "xs": np.ascontiguousarray(x[i * S:(i + 1) * S]),
            "bn1_w": bn1_w, "bn1_b": bn1_b,
            "bnc_w": bnc_w, "bnc_b": bnc_b,
        })
    res = run_bass_kernel_spmd(nc, in_maps, core_ids=list(range(NCORES)))
    global LAST_RESULT
    LAST_RESULT = res
    out = np.concatenate(
        [np.asarray(res.results[i]["outy"]).astype(np.float32)
         for i in range(NCORES)], axis=0)
    return out
